# revision 11
# baseline (speedup 1.0000x reference)
"""Trainium2 Bass kernel for nn_CMix_x060moe (RWKV CMix + hash-routed MoE).

Strategy: expert-sharded SPMD over 8 NeuronCores. Hash routing depends only
on token_ids, so the host computes the token->expert assignment as part of
sharding: core e receives exactly 2048 tokens (expert e's kept tokens in
FIFO order, padded with capacity-dropped tokens from anywhere, mask=0 for
those). The RWKV token-shift (xk/xr = x + (xprev-x)*maa) is folded into the
host-side gather, so the device receives xk/xr directly and every device
instruction is matmul roofline work. Each core computes the dense
squared-ReLU FFN, its own expert's FFN and the sigmoid receptance for its
2048 tokens; the host scatters rows back. No collectives needed and the
load is perfectly balanced.

All activations live C-major ("transposed", [C, tokens]) on device so every
matmul keeps weights as the stationary operand. Weights and activations are
bf16 (full PE rate, LDWEIGHTS fully hidden, half the HBM traffic of f32);
PSUM accumulation stays f32. Weights are streamed exactly once: the token
block is processed in a single pass with the dense second-layer contraction
split into 2 groups so the hidden activations fit SBUF.
"""

import os

import ml_dtypes
import numpy as np

import concourse.mybir as mybir
import concourse.tile as tile
from concourse import bacc
from concourse.bass_utils import run_bass_kernel_spmd

LAST_RESULTS = None  # set on every kernel() call; holds BassKernelResults

B, T, C = 8, 2048, 1024
DFF, DFFE = 4096, 2048
E = 8
HASH_PRIME = 5099
CAP = (B * T) // E  # 2048 tokens per core
N = B * T

P = 128               # partitions
TW = 512              # matmul token width (one f32 psum bank)
NT = CAP // TW        # 4 token blocks
CT = C // P           # 8  C-tiles
MT_D = DFF // P       # 32 dense-hidden tiles
MT_E = DFFE // P      # 16 expert-hidden tiles
GD = 2                # dense second-layer contraction groups
HD = MT_D // GD       # 16 k-tiles per dense group

F32 = mybir.dt.float32
BF16 = mybir.dt.bfloat16

_COMPILED = None


def _dma_chunked(nc, dst, src, width, chunk):
    """Split a wide weight DMA into column chunks so each rides its own
    HWDGE queue (single-queue BW is ~1/16th of aggregate)."""
    for o in range(0, width, chunk):
        e = min(o + chunk, width)
        nc.sync.dma_start(dst[:, o:e], src[:, o:e])


def _build():
    nc = bacc.Bacc(trn_type="TRN2")

    xk_d = nc.dram_tensor("xk", [CT, P, CAP], BF16, kind="ExternalInput")
    xr_d = nc.dram_tensor("xr", [CT, P, CAP], BF16, kind="ExternalInput")
    maskd = nc.dram_tensor("maskd", [P, CAP], BF16, kind="ExternalInput")
    # weights, host-tiled p-major: w*[m][p][k*P+q] = W[k*P+p, m*P+q]
    wk = nc.dram_tensor("wk", [MT_D, P, CT * P], BF16, kind="ExternalInput")
    wv = nc.dram_tensor("wv", [CT, P, MT_D * P], BF16, kind="ExternalInput")
    wr = nc.dram_tensor("wr", [CT, P, CT * P], BF16, kind="ExternalInput")
    wek = nc.dram_tensor("wek", [MT_E, P, CT * P], BF16, kind="ExternalInput")
    wev = nc.dram_tensor("wev", [CT, P, MT_E * P], BF16, kind="ExternalInput")
    yout = nc.dram_tensor("y", [CT, P, CAP], BF16, kind="ExternalOutput")

    with tile.TileContext(nc) as tc:
        with (
            tc.tile_pool(name="const", bufs=1) as constp,
            tc.tile_pool(name="acts", bufs=1) as acts,
            tc.tile_pool(name="wfirst", bufs=3) as wfp,
            tc.tile_pool(name="wsecond", bufs=3) as wsp,
            tc.tile_pool(name="tmp", bufs=3) as tmpp,
            tc.tile_pool(name="outp", bufs=3) as outp,
            tc.tile_pool(name="ps1", bufs=3, space="PSUM") as ps1,
            tc.tile_pool(name="ps2", bufs=3, space="PSUM") as ps2,
            tc.tile_pool(name="psr", bufs=2, space="PSUM") as psr,
        ):
            tbs = [slice(t * TW, (t + 1) * TW) for t in range(NT)]

            # persistent activations (bf16): xk/xr inputs, kv accumulator
            xkt = [acts.tile([P, CAP], BF16, tag=f"xk{c}", name=f"xk{c}")
                   for c in range(CT)]
            xrt = [acts.tile([P, CAP], BF16, tag=f"xr{c}", name=f"xr{c}")
                   for c in range(CT)]
            kvt = [acts.tile([P, CAP], BF16, tag=f"kv{c}", name=f"kv{c}")
                   for c in range(CT)]

            # DMA issue order tracks first-use order (queues are FIFO):
            # xk tb0, first expert weight, xk tb1, second weight, xk rest.
            # xr is NOT issued here — it would head-of-line-block the
            # in-loop wek fetches; it goes out after the expert-L1 loop.
            for cc in range(CT):
                nc.sync.dma_start(xkt[cc][:, tbs[0]], xk_d[cc, :, tbs[0]])
            wek_tiles = {}
            # 4 chunks: they land on the 8 queues xk tb0 is not using, so
            # the first weight rides in parallel with the first activations
            wt = wfp.tile([P, CT * P], BF16, tag="w1")
            _dma_chunked(nc, wt, wek[0], CT * P, 256)
            wek_tiles[0] = wt
            wt = wfp.tile([P, CT * P], BF16, tag="w1")
            _dma_chunked(nc, wt, wek[1], CT * P, 256)
            wek_tiles[1] = wt
            for cc in range(CT):
                nc.sync.dma_start(xkt[cc][:, tbs[1]], xk_d[cc, :, tbs[1]])
            for t in range(2, NT):
                for cc in range(CT):
                    nc.sync.dma_start(xkt[cc][:, tbs[t]], xk_d[cc, :, tbs[t]])
            tmask = constp.tile([P, CAP], BF16)
            nc.sync.dma_start(tmask[:], maskd[:])

            def l1_phase(wdram, m0, mts, xtiles, pre):
                """hidden[i] = relu(x @ W[m0+i])^2 for i in range(mts), bf16."""
                out = []
                for i in range(mts):
                    m = m0 + i
                    if m in pre:
                        wt = pre[m]
                    else:
                        wt = wfp.tile([P, CT * P], BF16, tag="w1")
                        _dma_chunked(nc, wt, wdram[m], CT * P, 512)
                    ht = acts.tile([P, CAP], BF16, tag=f"h{i}", name=f"h{i}")
                    out.append(ht)
                    for t in range(NT):
                        pd = ps1.tile([P, TW], F32, tag="ps1")
                        for k in range(CT):
                            nc.tensor.matmul(
                                pd[:], wt[:, k * P:(k + 1) * P],
                                xtiles[k][:, tbs[t]],
                                start=(k == 0), stop=(k == CT - 1),
                            )
                        rl = tmpp.tile([P, TW], BF16, tag="rl")
                        nc.scalar.activation(
                            rl[:], pd[:], mybir.ActivationFunctionType.Relu
                        )
                        nc.vector.tensor_tensor(
                            out=ht[:, tbs[t]], in0=rl[:], in1=rl[:],
                            op=mybir.AluOpType.mult,
                        )
                return out

            def l2_chain(po, wt, htiles, t, nk):
                for k in range(nk):
                    nc.tensor.matmul(
                        po[:], wt[:, k * P:(k + 1) * P],
                        htiles[k][:, tbs[t]],
                        start=(k == 0), stop=(k == nk - 1),
                    )

            # ---- expert FFN first: kv = mask * (relu(xk@Wek)^2 @ Wev) ----
            ht = l1_phase(wek, 0, MT_E, xkt, wek_tiles)
            # xr is needed only by the receptance at the very end; issue
            # its DMAs here so they ride behind the expert-L1 weights.
            for t in range(NT):
                for cc in range(CT):
                    nc.sync.dma_start(xrt[cc][:, tbs[t]], xr_d[cc, :, tbs[t]])
            for c in range(CT):
                wt = wsp.tile([P, MT_E * P], BF16, tag="w2")
                _dma_chunked(nc, wt, wev[c], MT_E * P, 512)
                for t in range(NT):
                    po = ps2.tile([P, TW], F32, tag="ps2")
                    l2_chain(po, wt, ht, t, MT_E)
                    nc.vector.tensor_tensor(
                        out=kvt[c][:, tbs[t]], in0=po[:], in1=tmask[:, tbs[t]],
                        op=mybir.AluOpType.mult,
                    )

            # ---- dense FFN: kv += relu(xk@Wk)^2 @ Wv, 2 k-groups ----
            for g in range(GD):
                kt = l1_phase(wk, g * HD, HD, xkt, {})
                last_group = g == GD - 1
                for c in range(CT):
                    wt = wsp.tile([P, HD * P], BF16, tag="w2")
                    _dma_chunked(
                        nc, wt, wv[c, :, g * HD * P:(g + 1) * HD * P],
                        HD * P, 512,
                    )
                    for t in range(NT):
                        pv = ps2.tile([P, TW], F32, tag="ps2")
                        l2_chain(pv, wt, kt, t, HD)
                        nc.vector.tensor_tensor(
                            out=kvt[c][:, tbs[t]], in0=pv[:],
                            in1=kvt[c][:, tbs[t]],
                            op=mybir.AluOpType.add,
                        )
                    if last_group:
                        # ---- receptance per c-tile as soon as kv[c] final:
                        # y = sigmoid(xr @ Wr) * kv
                        wrt = wfp.tile([P, CT * P], BF16, tag="w1")
                        _dma_chunked(nc, wrt, wr[c], CT * P, 512)
                        for t in range(NT):
                            pr = psr.tile([P, TW], F32, tag="psr")
                            for k in range(CT):
                                nc.tensor.matmul(
                                    pr[:], wrt[:, k * P:(k + 1) * P],
                                    xrt[k][:, tbs[t]],
                                    start=(k == 0), stop=(k == CT - 1),
                                )
                            rm = tmpp.tile([P, TW], BF16, tag="rm")
                            nc.scalar.activation(
                                rm[:], pr[:],
                                mybir.ActivationFunctionType.Sigmoid,
                            )
                            yo = outp.tile([P, TW], BF16, tag="yo")
                            nc.vector.tensor_tensor(
                                out=yo[:], in0=kvt[c][:, tbs[t]], in1=rm[:],
                                op=mybir.AluOpType.mult,
                            )
                            nc.sync.dma_start(yout[c, :, tbs[t]], yo[:])

    nc.compile()
    return nc


def _routing(token_ids: np.ndarray):
    """Token -> (per-core global token list [E, CAP], per-core keep mask)."""
    tid = token_ids.reshape(N).astype(np.int64)
    eidx = (tid * HASH_PRIME) % E
    order = np.argsort(eidx, kind="stable")  # FIFO within expert
    counts = np.bincount(eidx, minlength=E)
    starts = np.zeros(E + 1, np.int64)
    np.cumsum(counts, out=starts[1:])

    token_lists = np.empty((E, CAP), np.int64)
    masks = np.zeros((E, CAP), np.float32)
    dropped = []
    fill_needed = []
    for e in range(E):
        grp = order[starts[e]:starts[e + 1]]
        nk = min(len(grp), CAP)
        token_lists[e, :nk] = grp[:nk]
        masks[e, :nk] = 1.0
        dropped.append(grp[CAP:])
        fill_needed.append(CAP - nk)
    dropped = (
        np.concatenate(dropped) if dropped else np.empty(0, np.int64)
    )
    pos = 0
    for e in range(E):
        need = fill_needed[e]
        if need:
            token_lists[e, CAP - need:] = dropped[pos:pos + need]
            pos += need
    assert pos == len(dropped)
    return token_lists, masks


def _tile_first(W, mt):
    """[C, M] -> [mt, P, CT*P] with w[m][p][k*P+q] = W[k*P+p, m*P+q]."""
    ct = W.shape[0] // P
    return np.ascontiguousarray(
        W.reshape(ct, P, mt, P).transpose(2, 1, 0, 3).reshape(mt, P, ct * P)
    )


def _tile_second(W, ct_out):
    """[K, M] -> [ct_out, P, KT*P] with w[m][p][k*P+q] = W[k*P+p, m*P+q]."""
    kt = W.shape[0] // P
    return np.ascontiguousarray(
        W.reshape(kt, P, ct_out, P).transpose(2, 1, 0, 3).reshape(ct_out, P, kt * P)
    )


def kernel(x, shift_state, token_ids, time_maa_k, time_maa_r, Wk, Wv, Wr, Wek, Wev):
    global _COMPILED
    if _COMPILED is None:
        _COMPILED = _build()
    nc = _COMPILED

    x = np.asarray(x, np.float32)
    shift_state = np.asarray(shift_state, np.float32)
    token_lists, masks = _routing(np.asarray(token_ids))

    xf = x.reshape(N, C)
    xprev_f = np.empty_like(xf)
    xprev_f[1:] = xf[:-1]
    xprev_f[np.arange(B) * T] = shift_state

    # token shift on host: xk/xr = x + (xprev - x) * maa
    dx = xprev_f - xf
    maak = np.asarray(time_maa_k, np.float32)
    maar = np.asarray(time_maa_r, np.float32)
    xk_full = xf + dx * maak
    xr_full = xf + dx * maar

    bf = ml_dtypes.bfloat16
    wk_t = _tile_first(np.asarray(Wk, np.float32), MT_D).astype(bf)
    wr_t = _tile_first(np.asarray(Wr, np.float32), CT).astype(bf)
    wv_t = _tile_second(np.asarray(Wv, np.float32), CT).astype(bf)
    Wek = np.asarray(Wek, np.float32)
    Wev = np.asarray(Wev, np.float32)

    def ctmajor(rows):  # [CAP, C] -> [CT, P, CAP] bf16
        return np.ascontiguousarray(rows.T.reshape(CT, P, CAP)).astype(bf)

    in_maps = []
    for e in range(E):
        L = token_lists[e]
        in_maps.append(dict(
            xk=ctmajor(xk_full[L]),
            xr=ctmajor(xr_full[L]),
            maskd=np.ascontiguousarray(
                np.broadcast_to(masks[e], (P, CAP))
            ).astype(bf),
            wk=wk_t,
            wv=wv_t,
            wr=wr_t,
            wek=_tile_first(Wek[e], MT_E).astype(bf),
            wev=_tile_second(Wev[e], CT).astype(bf),
        ))

    res = run_bass_kernel_spmd(
        nc, in_maps, core_ids=list(range(E)),
        trace=bool(os.environ.get("KERNEL_TRACE")),
    )
    global LAST_RESULTS
    LAST_RESULTS = res

    y = np.empty((N, C), np.float32)
    for e in range(E):
        y[token_lists[e]] = (
            res.results[e]["y"].reshape(C, CAP).T.astype(np.float32)
        )
    return y.reshape(B, T, C)


# revision 12
# speedup vs baseline: 1.0012x; 1.0012x over previous
"""Trainium2 Bass kernel for nn_CMix_x060moe (RWKV CMix + hash-routed MoE).

Strategy: expert-sharded SPMD over 8 NeuronCores. Hash routing depends only
on token_ids, so the host computes the token->expert assignment as part of
sharding: core e receives exactly 2048 tokens (expert e's kept tokens in
FIFO order, padded with capacity-dropped tokens from anywhere, mask=0 for
those). The RWKV token-shift (xk/xr = x + (xprev-x)*maa) is folded into the
host-side gather, so the device receives xk/xr directly and every device
instruction is matmul roofline work. Each core computes the dense
squared-ReLU FFN, its own expert's FFN and the sigmoid receptance for its
2048 tokens; the host scatters rows back. No collectives needed and the
load is perfectly balanced.

All activations live C-major ("transposed", [C, tokens]) on device so every
matmul keeps weights as the stationary operand. Weights and activations are
bf16 (full PE rate, LDWEIGHTS fully hidden, half the HBM traffic of f32);
PSUM accumulation stays f32. Weights are streamed exactly once: the token
block is processed in a single pass with the dense second-layer contraction
split into 2 groups so the hidden activations fit SBUF.
"""

import os

import ml_dtypes
import numpy as np

import concourse.mybir as mybir
import concourse.tile as tile
from concourse import bacc
from concourse.bass_utils import run_bass_kernel_spmd

LAST_RESULTS = None  # set on every kernel() call; holds BassKernelResults

B, T, C = 8, 2048, 1024
DFF, DFFE = 4096, 2048
E = 8
HASH_PRIME = 5099
CAP = (B * T) // E  # 2048 tokens per core
N = B * T

P = 128               # partitions
TW = 512              # matmul token width (one f32 psum bank)
NT = CAP // TW        # 4 token blocks
CT = C // P           # 8  C-tiles
MT_D = DFF // P       # 32 dense-hidden tiles
MT_E = DFFE // P      # 16 expert-hidden tiles
GD = 2                # dense second-layer contraction groups
HD = MT_D // GD       # 16 k-tiles per dense group

F32 = mybir.dt.float32
BF16 = mybir.dt.bfloat16

_COMPILED = None


def _dma_chunked(nc, dst, src, width, chunk):
    """Split a wide weight DMA into column chunks so each rides its own
    HWDGE queue (single-queue BW is ~1/16th of aggregate)."""
    for o in range(0, width, chunk):
        e = min(o + chunk, width)
        nc.sync.dma_start(dst[:, o:e], src[:, o:e])


def _build():
    nc = bacc.Bacc(trn_type="TRN2")

    xk_d = nc.dram_tensor("xk", [CT, P, CAP], BF16, kind="ExternalInput")
    xr_d = nc.dram_tensor("xr", [CT, P, CAP], BF16, kind="ExternalInput")
    maskd = nc.dram_tensor("maskd", [P, CAP], BF16, kind="ExternalInput")
    # weights, host-tiled p-major: w*[m][p][k*P+q] = W[k*P+p, m*P+q]
    wk = nc.dram_tensor("wk", [MT_D, P, CT * P], BF16, kind="ExternalInput")
    wv = nc.dram_tensor("wv", [CT, P, MT_D * P], BF16, kind="ExternalInput")
    wr = nc.dram_tensor("wr", [CT, P, CT * P], BF16, kind="ExternalInput")
    wek = nc.dram_tensor("wek", [MT_E, P, CT * P], BF16, kind="ExternalInput")
    wev = nc.dram_tensor("wev", [CT, P, MT_E * P], BF16, kind="ExternalInput")
    yout = nc.dram_tensor("y", [CT, P, CAP], BF16, kind="ExternalOutput")

    with tile.TileContext(nc) as tc:
        with (
            tc.tile_pool(name="const", bufs=1) as constp,
            tc.tile_pool(name="acts", bufs=1) as acts,
            tc.tile_pool(name="wfirst", bufs=3) as wfp,
            tc.tile_pool(name="wsecond", bufs=3) as wsp,
            tc.tile_pool(name="tmp", bufs=3) as tmpp,
            tc.tile_pool(name="outp", bufs=3) as outp,
            tc.tile_pool(name="ps1", bufs=3, space="PSUM") as ps1,
            tc.tile_pool(name="ps2", bufs=3, space="PSUM") as ps2,
            tc.tile_pool(name="psr", bufs=2, space="PSUM") as psr,
        ):
            tbs = [slice(t * TW, (t + 1) * TW) for t in range(NT)]

            # persistent activations (bf16): xk/xr inputs, kv accumulator
            xkt = [acts.tile([P, CAP], BF16, tag=f"xk{c}", name=f"xk{c}")
                   for c in range(CT)]
            xrt = [acts.tile([P, CAP], BF16, tag=f"xr{c}", name=f"xr{c}")
                   for c in range(CT)]
            kvt = [acts.tile([P, CAP], BF16, tag=f"kv{c}", name=f"kv{c}")
                   for c in range(CT)]

            # DMA issue order tracks first-use order (queues are FIFO):
            # xk tb0, first expert weight, xk tb1, second weight, xk rest.
            # xr is NOT issued here — it would head-of-line-block the
            # in-loop wek fetches; it goes out after the expert-L1 loop.
            for cc in range(CT):
                nc.sync.dma_start(xkt[cc][:, tbs[0]], xk_d[cc, :, tbs[0]])
            wek_tiles = {}
            wt = wfp.tile([P, CT * P], BF16, tag="w1")
            _dma_chunked(nc, wt, wek[0], CT * P, 512)
            wek_tiles[0] = wt
            for cc in range(CT):
                nc.sync.dma_start(xkt[cc][:, tbs[1]], xk_d[cc, :, tbs[1]])
            wt = wfp.tile([P, CT * P], BF16, tag="w1")
            _dma_chunked(nc, wt, wek[1], CT * P, 512)
            wek_tiles[1] = wt
            for t in range(2, NT):
                for cc in range(CT):
                    nc.sync.dma_start(xkt[cc][:, tbs[t]], xk_d[cc, :, tbs[t]])
            tmask = constp.tile([P, CAP], BF16)
            nc.sync.dma_start(tmask[:], maskd[:])

            def l1_phase(wdram, m0, mts, xtiles, pre):
                """hidden[i] = relu(x @ W[m0+i])^2 for i in range(mts), bf16."""
                out = []
                for i in range(mts):
                    m = m0 + i
                    if m in pre:
                        wt = pre[m]
                    else:
                        wt = wfp.tile([P, CT * P], BF16, tag="w1")
                        _dma_chunked(nc, wt, wdram[m], CT * P, 512)
                    ht = acts.tile([P, CAP], BF16, tag=f"h{i}", name=f"h{i}")
                    out.append(ht)
                    for t in range(NT):
                        pd = ps1.tile([P, TW], F32, tag="ps1")
                        for k in range(CT):
                            nc.tensor.matmul(
                                pd[:], wt[:, k * P:(k + 1) * P],
                                xtiles[k][:, tbs[t]],
                                start=(k == 0), stop=(k == CT - 1),
                            )
                        rl = tmpp.tile([P, TW], BF16, tag="rl")
                        nc.scalar.activation(
                            rl[:], pd[:], mybir.ActivationFunctionType.Relu
                        )
                        nc.vector.tensor_tensor(
                            out=ht[:, tbs[t]], in0=rl[:], in1=rl[:],
                            op=mybir.AluOpType.mult,
                        )
                return out

            def l2_chain(po, wt, htiles, t, nk):
                for k in range(nk):
                    nc.tensor.matmul(
                        po[:], wt[:, k * P:(k + 1) * P],
                        htiles[k][:, tbs[t]],
                        start=(k == 0), stop=(k == nk - 1),
                    )

            # ---- expert FFN first: kv = mask * (relu(xk@Wek)^2 @ Wev) ----
            ht = l1_phase(wek, 0, MT_E, xkt, wek_tiles)
            # xr is needed only by the receptance at the very end; issue
            # its DMAs here so they ride behind the expert-L1 weights.
            for t in range(NT):
                for cc in range(CT):
                    nc.sync.dma_start(xrt[cc][:, tbs[t]], xr_d[cc, :, tbs[t]])
            for c in range(CT):
                wt = wsp.tile([P, MT_E * P], BF16, tag="w2")
                _dma_chunked(nc, wt, wev[c], MT_E * P, 512)
                for t in range(NT):
                    po = ps2.tile([P, TW], F32, tag="ps2")
                    l2_chain(po, wt, ht, t, MT_E)
                    nc.vector.tensor_tensor(
                        out=kvt[c][:, tbs[t]], in0=po[:], in1=tmask[:, tbs[t]],
                        op=mybir.AluOpType.mult,
                    )

            # ---- dense FFN: kv += relu(xk@Wk)^2 @ Wv, 2 k-groups ----
            for g in range(GD):
                kt = l1_phase(wk, g * HD, HD, xkt, {})
                last_group = g == GD - 1
                for c in range(CT):
                    wt = wsp.tile([P, HD * P], BF16, tag="w2")
                    _dma_chunked(
                        nc, wt, wv[c, :, g * HD * P:(g + 1) * HD * P],
                        HD * P, 512,
                    )
                    for t in range(NT):
                        pv = ps2.tile([P, TW], F32, tag="ps2")
                        l2_chain(pv, wt, kt, t, HD)
                        nc.vector.tensor_tensor(
                            out=kvt[c][:, tbs[t]], in0=pv[:],
                            in1=kvt[c][:, tbs[t]],
                            op=mybir.AluOpType.add,
                        )
                    if last_group:
                        # ---- receptance per c-tile as soon as kv[c] final:
                        # y = sigmoid(xr @ Wr) * kv
                        wrt = wfp.tile([P, CT * P], BF16, tag="w1")
                        _dma_chunked(nc, wrt, wr[c], CT * P, 512)
                        for t in range(NT):
                            pr = psr.tile([P, TW], F32, tag="psr")
                            for k in range(CT):
                                nc.tensor.matmul(
                                    pr[:], wrt[:, k * P:(k + 1) * P],
                                    xrt[k][:, tbs[t]],
                                    start=(k == 0), stop=(k == CT - 1),
                                )
                            rm = tmpp.tile([P, TW], BF16, tag="rm")
                            nc.scalar.activation(
                                rm[:], pr[:],
                                mybir.ActivationFunctionType.Sigmoid,
                            )
                            yo = outp.tile([P, TW], BF16, tag="yo")
                            nc.vector.tensor_tensor(
                                out=yo[:], in0=kvt[c][:, tbs[t]], in1=rm[:],
                                op=mybir.AluOpType.mult,
                            )
                            nc.sync.dma_start(yout[c, :, tbs[t]], yo[:])

    nc.compile()
    return nc


def _routing(token_ids: np.ndarray):
    """Token -> (per-core global token list [E, CAP], per-core keep mask)."""
    tid = token_ids.reshape(N).astype(np.int64)
    eidx = (tid * HASH_PRIME) % E
    order = np.argsort(eidx, kind="stable")  # FIFO within expert
    counts = np.bincount(eidx, minlength=E)
    starts = np.zeros(E + 1, np.int64)
    np.cumsum(counts, out=starts[1:])

    token_lists = np.empty((E, CAP), np.int64)
    masks = np.zeros((E, CAP), np.float32)
    dropped = []
    fill_needed = []
    for e in range(E):
        grp = order[starts[e]:starts[e + 1]]
        nk = min(len(grp), CAP)
        token_lists[e, :nk] = grp[:nk]
        masks[e, :nk] = 1.0
        dropped.append(grp[CAP:])
        fill_needed.append(CAP - nk)
    dropped = (
        np.concatenate(dropped) if dropped else np.empty(0, np.int64)
    )
    pos = 0
    for e in range(E):
        need = fill_needed[e]
        if need:
            token_lists[e, CAP - need:] = dropped[pos:pos + need]
            pos += need
    assert pos == len(dropped)
    return token_lists, masks


def _tile_first(W, mt):
    """[C, M] -> [mt, P, CT*P] with w[m][p][k*P+q] = W[k*P+p, m*P+q]."""
    ct = W.shape[0] // P
    return np.ascontiguousarray(
        W.reshape(ct, P, mt, P).transpose(2, 1, 0, 3).reshape(mt, P, ct * P)
    )


def _tile_second(W, ct_out):
    """[K, M] -> [ct_out, P, KT*P] with w[m][p][k*P+q] = W[k*P+p, m*P+q]."""
    kt = W.shape[0] // P
    return np.ascontiguousarray(
        W.reshape(kt, P, ct_out, P).transpose(2, 1, 0, 3).reshape(ct_out, P, kt * P)
    )


def kernel(x, shift_state, token_ids, time_maa_k, time_maa_r, Wk, Wv, Wr, Wek, Wev):
    global _COMPILED
    if _COMPILED is None:
        _COMPILED = _build()
    nc = _COMPILED

    x = np.asarray(x, np.float32)
    shift_state = np.asarray(shift_state, np.float32)
    token_lists, masks = _routing(np.asarray(token_ids))

    xf = x.reshape(N, C)
    xprev_f = np.empty_like(xf)
    xprev_f[1:] = xf[:-1]
    xprev_f[np.arange(B) * T] = shift_state

    # token shift on host: xk/xr = x + (xprev - x) * maa
    dx = xprev_f - xf
    maak = np.asarray(time_maa_k, np.float32)
    maar = np.asarray(time_maa_r, np.float32)
    xk_full = xf + dx * maak
    xr_full = xf + dx * maar

    bf = ml_dtypes.bfloat16
    wk_t = _tile_first(np.asarray(Wk, np.float32), MT_D).astype(bf)
    wr_t = _tile_first(np.asarray(Wr, np.float32), CT).astype(bf)
    wv_t = _tile_second(np.asarray(Wv, np.float32), CT).astype(bf)
    Wek = np.asarray(Wek, np.float32)
    Wev = np.asarray(Wev, np.float32)

    def ctmajor(rows):  # [CAP, C] -> [CT, P, CAP] bf16
        return np.ascontiguousarray(rows.T.reshape(CT, P, CAP)).astype(bf)

    in_maps = []
    for e in range(E):
        L = token_lists[e]
        in_maps.append(dict(
            xk=ctmajor(xk_full[L]),
            xr=ctmajor(xr_full[L]),
            maskd=np.ascontiguousarray(
                np.broadcast_to(masks[e], (P, CAP))
            ).astype(bf),
            wk=wk_t,
            wv=wv_t,
            wr=wr_t,
            wek=_tile_first(Wek[e], MT_E).astype(bf),
            wev=_tile_second(Wev[e], CT).astype(bf),
        ))

    res = run_bass_kernel_spmd(
        nc, in_maps, core_ids=list(range(E)),
        trace=bool(os.environ.get("KERNEL_TRACE")),
    )
    global LAST_RESULTS
    LAST_RESULTS = res

    y = np.empty((N, C), np.float32)
    for e in range(E):
        y[token_lists[e]] = (
            res.results[e]["y"].reshape(C, CAP).T.astype(np.float32)
        )
    return y.reshape(B, T, C)


# revision 15
# speedup vs baseline: 1.0016x; 1.0004x over previous
"""Trainium2 Bass kernel for nn_CMix_x060moe (RWKV CMix + hash-routed MoE).

Strategy: expert-sharded SPMD over 8 NeuronCores. Hash routing depends only
on token_ids, so the host computes the token->expert assignment as part of
sharding: core e receives exactly 2048 tokens (expert e's kept tokens in
FIFO order, padded with capacity-dropped tokens from anywhere, mask=0 for
those). The RWKV token-shift (xk/xr = x + (xprev-x)*maa) is folded into the
host-side gather, so the device receives xk/xr directly and every device
instruction is matmul roofline work. Each core computes the dense
squared-ReLU FFN, its own expert's FFN and the sigmoid receptance for its
2048 tokens; the host scatters rows back. No collectives needed and the
load is perfectly balanced.

All activations live C-major ("transposed", [C, tokens]) on device so every
matmul keeps weights as the stationary operand. Weights and activations are
bf16 (full PE rate, LDWEIGHTS fully hidden, half the HBM traffic of f32);
PSUM accumulation stays f32. Weights are streamed exactly once: the token
block is processed in a single pass with the dense second-layer contraction
split into 2 groups so the hidden activations fit SBUF.
"""

import os

import ml_dtypes
import numpy as np

import concourse.mybir as mybir
import concourse.tile as tile
from concourse import bacc
from concourse.bass_utils import run_bass_kernel_spmd

LAST_RESULTS = None  # set on every kernel() call; holds BassKernelResults

B, T, C = 8, 2048, 1024
DFF, DFFE = 4096, 2048
E = 8
HASH_PRIME = 5099
CAP = (B * T) // E  # 2048 tokens per core
N = B * T

P = 128               # partitions
TW = 512              # matmul token width (one f32 psum bank)
NT = CAP // TW        # 4 token blocks
CT = C // P           # 8  C-tiles
MT_D = DFF // P       # 32 dense-hidden tiles
MT_E = DFFE // P      # 16 expert-hidden tiles
GD = 2                # dense second-layer contraction groups
HD = MT_D // GD       # 16 k-tiles per dense group

F32 = mybir.dt.float32
BF16 = mybir.dt.bfloat16

_COMPILED = None


def _dma_chunked(nc, dst, src, width, chunk):
    """Split a wide weight DMA into column chunks so each rides its own
    HWDGE queue (single-queue BW is ~1/16th of aggregate)."""
    for o in range(0, width, chunk):
        e = min(o + chunk, width)
        nc.sync.dma_start(dst[:, o:e], src[:, o:e])


def _build():
    nc = bacc.Bacc(trn_type="TRN2")

    xk_d = nc.dram_tensor("xk", [CT, P, CAP], BF16, kind="ExternalInput")
    xr_d = nc.dram_tensor("xr", [CT, P, CAP], BF16, kind="ExternalInput")
    maskd = nc.dram_tensor("maskd", [P, CAP], BF16, kind="ExternalInput")
    # weights, host-tiled p-major: w*[m][p][k*P+q] = W[k*P+p, m*P+q]
    wk = nc.dram_tensor("wk", [MT_D, P, CT * P], BF16, kind="ExternalInput")
    wv = nc.dram_tensor("wv", [CT, P, MT_D * P], BF16, kind="ExternalInput")
    wr = nc.dram_tensor("wr", [CT, P, CT * P], BF16, kind="ExternalInput")
    wek = nc.dram_tensor("wek", [MT_E, P, CT * P], BF16, kind="ExternalInput")
    wev = nc.dram_tensor("wev", [CT, P, MT_E * P], BF16, kind="ExternalInput")
    yout = nc.dram_tensor("y", [CT, P, CAP], BF16, kind="ExternalOutput")

    with tile.TileContext(nc) as tc:
        with (
            tc.tile_pool(name="const", bufs=1) as constp,
            tc.tile_pool(name="acts", bufs=1) as acts,
            tc.tile_pool(name="wfirst", bufs=3) as wfp,
            tc.tile_pool(name="wsecond", bufs=3) as wsp,
            tc.tile_pool(name="tmp", bufs=3) as tmpp,
            tc.tile_pool(name="outp", bufs=3) as outp,
            tc.tile_pool(name="ps1", bufs=3, space="PSUM") as ps1,
            tc.tile_pool(name="ps2", bufs=3, space="PSUM") as ps2,
            tc.tile_pool(name="psr", bufs=2, space="PSUM") as psr,
        ):
            tbs = [slice(t * TW, (t + 1) * TW) for t in range(NT)]

            # PE warm-up: the HAM clock gate holds the PE at 1.2 GHz until
            # ~3.4us of sustained activity. Run dummy matmuls on zeroed
            # tiles during the otherwise-idle input-DMA window so the real
            # chains start at 2.4 GHz. Sized to end just before the first
            # xk/wek data lands (~15us).
            dw = constp.tile([P, P], BF16, name="dummy_w")
            nc.vector.memzero(dw[:])
            da = constp.tile([P, TW], BF16, name="dummy_a")
            nc.vector.memzero(da[:])
            dps = ps1.tile([P, TW], F32, tag="ps1")
            NWARM = 56
            for i in range(NWARM):
                nc.tensor.matmul(dps[:], dw[:], da[:],
                                 start=(i == 0), stop=(i == NWARM - 1))

            # persistent activations (bf16): xk/xr inputs, kv accumulator
            xkt = [acts.tile([P, CAP], BF16, tag=f"xk{c}", name=f"xk{c}")
                   for c in range(CT)]
            xrt = [acts.tile([P, CAP], BF16, tag=f"xr{c}", name=f"xr{c}")
                   for c in range(CT)]
            kvt = [acts.tile([P, CAP], BF16, tag=f"kv{c}", name=f"kv{c}")
                   for c in range(CT)]

            # DMA issue order tracks first-use order (queues are FIFO):
            # xk tb0, first expert weight, xk tb1, second weight, xk rest.
            # xr is NOT issued here — it would head-of-line-block the
            # in-loop wek fetches; it goes out after the expert-L1 loop.
            for cc in range(CT):
                nc.sync.dma_start(xkt[cc][:, tbs[0]], xk_d[cc, :, tbs[0]])
            wek_tiles = {}
            wt = wfp.tile([P, CT * P], BF16, tag="w1")
            _dma_chunked(nc, wt, wek[0], CT * P, 512)
            wek_tiles[0] = wt
            for cc in range(CT):
                nc.sync.dma_start(xkt[cc][:, tbs[1]], xk_d[cc, :, tbs[1]])
            wt = wfp.tile([P, CT * P], BF16, tag="w1")
            _dma_chunked(nc, wt, wek[1], CT * P, 512)
            wek_tiles[1] = wt
            for t in range(2, NT):
                for cc in range(CT):
                    nc.sync.dma_start(xkt[cc][:, tbs[t]], xk_d[cc, :, tbs[t]])
            tmask = constp.tile([P, CAP], BF16)
            nc.sync.dma_start(tmask[:], maskd[:])

            def l1_phase(wdram, m0, mts, xtiles, pre):
                """hidden[i] = relu(x @ W[m0+i])^2 for i in range(mts), bf16."""
                out = []
                for i in range(mts):
                    m = m0 + i
                    if m in pre:
                        wt = pre[m]
                    else:
                        wt = wfp.tile([P, CT * P], BF16, tag="w1")
                        _dma_chunked(nc, wt, wdram[m], CT * P, 512)
                    ht = acts.tile([P, CAP], BF16, tag=f"h{i}", name=f"h{i}")
                    out.append(ht)
                    for t in range(NT):
                        pd = ps1.tile([P, TW], F32, tag="ps1")
                        for k in range(CT):
                            nc.tensor.matmul(
                                pd[:], wt[:, k * P:(k + 1) * P],
                                xtiles[k][:, tbs[t]],
                                start=(k == 0), stop=(k == CT - 1),
                            )
                        rl = tmpp.tile([P, TW], BF16, tag="rl")
                        nc.scalar.activation(
                            rl[:], pd[:], mybir.ActivationFunctionType.Relu
                        )
                        nc.vector.tensor_tensor(
                            out=ht[:, tbs[t]], in0=rl[:], in1=rl[:],
                            op=mybir.AluOpType.mult,
                        )
                return out

            def l2_chain(po, wt, htiles, t, nk):
                for k in range(nk):
                    nc.tensor.matmul(
                        po[:], wt[:, k * P:(k + 1) * P],
                        htiles[k][:, tbs[t]],
                        start=(k == 0), stop=(k == nk - 1),
                    )

            # ---- expert FFN first: kv = mask * (relu(xk@Wek)^2 @ Wev) ----
            ht = l1_phase(wek, 0, MT_E, xkt, wek_tiles)
            # xr is needed only by the receptance at the very end; issue
            # its DMAs here so they ride behind the expert-L1 weights.
            for t in range(NT):
                for cc in range(CT):
                    nc.sync.dma_start(xrt[cc][:, tbs[t]], xr_d[cc, :, tbs[t]])
            for c in range(CT):
                wt = wsp.tile([P, MT_E * P], BF16, tag="w2")
                _dma_chunked(nc, wt, wev[c], MT_E * P, 512)
                for t in range(NT):
                    po = ps2.tile([P, TW], F32, tag="ps2")
                    l2_chain(po, wt, ht, t, MT_E)
                    nc.vector.tensor_tensor(
                        out=kvt[c][:, tbs[t]], in0=po[:], in1=tmask[:, tbs[t]],
                        op=mybir.AluOpType.mult,
                    )

            # ---- dense FFN: kv += relu(xk@Wk)^2 @ Wv, 2 k-groups ----
            for g in range(GD):
                kt = l1_phase(wk, g * HD, HD, xkt, {})
                last_group = g == GD - 1
                for c in range(CT):
                    wt = wsp.tile([P, HD * P], BF16, tag="w2")
                    _dma_chunked(
                        nc, wt, wv[c, :, g * HD * P:(g + 1) * HD * P],
                        HD * P, 512,
                    )
                    for t in range(NT):
                        pv = ps2.tile([P, TW], F32, tag="ps2")
                        l2_chain(pv, wt, kt, t, HD)
                        nc.vector.tensor_tensor(
                            out=kvt[c][:, tbs[t]], in0=pv[:],
                            in1=kvt[c][:, tbs[t]],
                            op=mybir.AluOpType.add,
                        )
                    if last_group:
                        # ---- receptance per c-tile as soon as kv[c] final:
                        # y = sigmoid(xr @ Wr) * kv
                        wrt = wfp.tile([P, CT * P], BF16, tag="w1")
                        _dma_chunked(nc, wrt, wr[c], CT * P, 512)
                        for t in range(NT):
                            pr = psr.tile([P, TW], F32, tag="psr")
                            for k in range(CT):
                                nc.tensor.matmul(
                                    pr[:], wrt[:, k * P:(k + 1) * P],
                                    xrt[k][:, tbs[t]],
                                    start=(k == 0), stop=(k == CT - 1),
                                )
                            rm = tmpp.tile([P, TW], BF16, tag="rm")
                            nc.scalar.activation(
                                rm[:], pr[:],
                                mybir.ActivationFunctionType.Sigmoid,
                            )
                            yo = outp.tile([P, TW], BF16, tag="yo")
                            nc.vector.tensor_tensor(
                                out=yo[:], in0=kvt[c][:, tbs[t]], in1=rm[:],
                                op=mybir.AluOpType.mult,
                            )
                            nc.sync.dma_start(yout[c, :, tbs[t]], yo[:])

    nc.compile()
    return nc


def _routing(token_ids: np.ndarray):
    """Token -> (per-core global token list [E, CAP], per-core keep mask)."""
    tid = token_ids.reshape(N).astype(np.int64)
    eidx = (tid * HASH_PRIME) % E
    order = np.argsort(eidx, kind="stable")  # FIFO within expert
    counts = np.bincount(eidx, minlength=E)
    starts = np.zeros(E + 1, np.int64)
    np.cumsum(counts, out=starts[1:])

    token_lists = np.empty((E, CAP), np.int64)
    masks = np.zeros((E, CAP), np.float32)
    dropped = []
    fill_needed = []
    for e in range(E):
        grp = order[starts[e]:starts[e + 1]]
        nk = min(len(grp), CAP)
        token_lists[e, :nk] = grp[:nk]
        masks[e, :nk] = 1.0
        dropped.append(grp[CAP:])
        fill_needed.append(CAP - nk)
    dropped = (
        np.concatenate(dropped) if dropped else np.empty(0, np.int64)
    )
    pos = 0
    for e in range(E):
        need = fill_needed[e]
        if need:
            token_lists[e, CAP - need:] = dropped[pos:pos + need]
            pos += need
    assert pos == len(dropped)
    return token_lists, masks


def _tile_first(W, mt):
    """[C, M] -> [mt, P, CT*P] with w[m][p][k*P+q] = W[k*P+p, m*P+q]."""
    ct = W.shape[0] // P
    return np.ascontiguousarray(
        W.reshape(ct, P, mt, P).transpose(2, 1, 0, 3).reshape(mt, P, ct * P)
    )


def _tile_second(W, ct_out):
    """[K, M] -> [ct_out, P, KT*P] with w[m][p][k*P+q] = W[k*P+p, m*P+q]."""
    kt = W.shape[0] // P
    return np.ascontiguousarray(
        W.reshape(kt, P, ct_out, P).transpose(2, 1, 0, 3).reshape(ct_out, P, kt * P)
    )


def kernel(x, shift_state, token_ids, time_maa_k, time_maa_r, Wk, Wv, Wr, Wek, Wev):
    global _COMPILED
    if _COMPILED is None:
        _COMPILED = _build()
    nc = _COMPILED

    x = np.asarray(x, np.float32)
    shift_state = np.asarray(shift_state, np.float32)
    token_lists, masks = _routing(np.asarray(token_ids))

    xf = x.reshape(N, C)
    xprev_f = np.empty_like(xf)
    xprev_f[1:] = xf[:-1]
    xprev_f[np.arange(B) * T] = shift_state

    # token shift on host: xk/xr = x + (xprev - x) * maa
    dx = xprev_f - xf
    maak = np.asarray(time_maa_k, np.float32)
    maar = np.asarray(time_maa_r, np.float32)
    xk_full = xf + dx * maak
    xr_full = xf + dx * maar

    bf = ml_dtypes.bfloat16
    wk_t = _tile_first(np.asarray(Wk, np.float32), MT_D).astype(bf)
    wr_t = _tile_first(np.asarray(Wr, np.float32), CT).astype(bf)
    wv_t = _tile_second(np.asarray(Wv, np.float32), CT).astype(bf)
    Wek = np.asarray(Wek, np.float32)
    Wev = np.asarray(Wev, np.float32)

    def ctmajor(rows):  # [CAP, C] -> [CT, P, CAP] bf16
        return np.ascontiguousarray(rows.T.reshape(CT, P, CAP)).astype(bf)

    in_maps = []
    for e in range(E):
        L = token_lists[e]
        in_maps.append(dict(
            xk=ctmajor(xk_full[L]),
            xr=ctmajor(xr_full[L]),
            maskd=np.ascontiguousarray(
                np.broadcast_to(masks[e], (P, CAP))
            ).astype(bf),
            wk=wk_t,
            wv=wv_t,
            wr=wr_t,
            wek=_tile_first(Wek[e], MT_E).astype(bf),
            wev=_tile_second(Wev[e], CT).astype(bf),
        ))

    res = run_bass_kernel_spmd(
        nc, in_maps, core_ids=list(range(E)),
        trace=bool(os.environ.get("KERNEL_TRACE")),
    )
    global LAST_RESULTS
    LAST_RESULTS = res

    y = np.empty((N, C), np.float32)
    for e in range(E):
        y[token_lists[e]] = (
            res.results[e]["y"].reshape(C, CAP).T.astype(np.float32)
        )
    return y.reshape(B, T, C)


# revision 16
# speedup vs baseline: 1.0031x; 1.0015x over previous
"""Trainium2 Bass kernel for nn_CMix_x060moe (RWKV CMix + hash-routed MoE).

Strategy: expert-sharded SPMD over 8 NeuronCores. Hash routing depends only
on token_ids, so the host computes the token->expert assignment as part of
sharding: core e receives exactly 2048 tokens (expert e's kept tokens in
FIFO order, padded with capacity-dropped tokens from anywhere, mask=0 for
those). The RWKV token-shift (xk/xr = x + (xprev-x)*maa) is folded into the
host-side gather, so the device receives xk/xr directly and every device
instruction is matmul roofline work. Each core computes the dense
squared-ReLU FFN, its own expert's FFN and the sigmoid receptance for its
2048 tokens; the host scatters rows back. No collectives needed and the
load is perfectly balanced.

All activations live C-major ("transposed", [C, tokens]) on device so every
matmul keeps weights as the stationary operand. Weights and activations are
bf16 (full PE rate, LDWEIGHTS fully hidden, half the HBM traffic of f32);
PSUM accumulation stays f32. Weights are streamed exactly once: the token
block is processed in a single pass with the dense second-layer contraction
split into 2 groups so the hidden activations fit SBUF.
"""

import os

import ml_dtypes
import numpy as np

import concourse.mybir as mybir
import concourse.tile as tile
from concourse import bacc
from concourse.bass_utils import run_bass_kernel_spmd

LAST_RESULTS = None  # set on every kernel() call; holds BassKernelResults

B, T, C = 8, 2048, 1024
DFF, DFFE = 4096, 2048
E = 8
HASH_PRIME = 5099
CAP = (B * T) // E  # 2048 tokens per core
N = B * T

P = 128               # partitions
TW = 512              # matmul token width (one f32 psum bank)
NT = CAP // TW        # 4 token blocks
CT = C // P           # 8  C-tiles
MT_D = DFF // P       # 32 dense-hidden tiles
MT_E = DFFE // P      # 16 expert-hidden tiles
GD = 2                # dense second-layer contraction groups
HD = MT_D // GD       # 16 k-tiles per dense group

F32 = mybir.dt.float32
BF16 = mybir.dt.bfloat16

_COMPILED = None


def _dma_chunked(nc, dst, src, width, chunk):
    """Split a wide weight DMA into column chunks so each rides its own
    HWDGE queue (single-queue BW is ~1/16th of aggregate)."""
    for o in range(0, width, chunk):
        e = min(o + chunk, width)
        nc.sync.dma_start(dst[:, o:e], src[:, o:e])


def _build():
    nc = bacc.Bacc(trn_type="TRN2")

    xk_d = nc.dram_tensor("xk", [CT, P, CAP], BF16, kind="ExternalInput")
    xr_d = nc.dram_tensor("xr", [CT, P, CAP], BF16, kind="ExternalInput")
    maskd = nc.dram_tensor("maskd", [P, CAP], BF16, kind="ExternalInput")
    # weights, host-tiled p-major: w*[m][p][k*P+q] = W[k*P+p, m*P+q]
    wk = nc.dram_tensor("wk", [MT_D, P, CT * P], BF16, kind="ExternalInput")
    wv = nc.dram_tensor("wv", [CT, P, MT_D * P], BF16, kind="ExternalInput")
    wr = nc.dram_tensor("wr", [CT, P, CT * P], BF16, kind="ExternalInput")
    wek = nc.dram_tensor("wek", [MT_E, P, CT * P], BF16, kind="ExternalInput")
    wev = nc.dram_tensor("wev", [CT, P, MT_E * P], BF16, kind="ExternalInput")
    yout = nc.dram_tensor("y", [CT, P, CAP], BF16, kind="ExternalOutput")

    with tile.TileContext(nc) as tc:
        with (
            tc.tile_pool(name="const", bufs=1) as constp,
            tc.tile_pool(name="acts", bufs=1) as acts,
            tc.tile_pool(name="wfirst", bufs=3) as wfp,
            tc.tile_pool(name="wsecond", bufs=3) as wsp,
            tc.tile_pool(name="tmp", bufs=3) as tmpp,
            tc.tile_pool(name="outp", bufs=3) as outp,
            tc.tile_pool(name="ps1", bufs=3, space="PSUM") as ps1,
            tc.tile_pool(name="ps2", bufs=3, space="PSUM") as ps2,
            tc.tile_pool(name="psr", bufs=2, space="PSUM") as psr,
        ):
            tbs = [slice(t * TW, (t + 1) * TW) for t in range(NT)]

            # PE warm-up: the HAM clock gate holds the PE at 1.2 GHz until
            # ~3.4us of sustained activity. Run dummy matmuls on zeroed
            # tiles during the otherwise-idle input-DMA window so the real
            # chains start at 2.4 GHz. Sized to end just before the first
            # xk/wek data lands (~15us).
            dw = constp.tile([P, P], BF16, name="dummy_w")
            nc.vector.memzero(dw[:])
            da = constp.tile([P, TW], BF16, name="dummy_a")
            nc.vector.memzero(da[:])
            dps = ps1.tile([P, TW], F32, tag="ps1")
            NWARM = 28
            for i in range(NWARM):
                nc.tensor.matmul(dps[:], dw[:], da[:],
                                 start=(i == 0), stop=(i == NWARM - 1))

            # persistent activations (bf16): xk/xr inputs, kv accumulator
            xkt = [acts.tile([P, CAP], BF16, tag=f"xk{c}", name=f"xk{c}")
                   for c in range(CT)]
            xrt = [acts.tile([P, CAP], BF16, tag=f"xr{c}", name=f"xr{c}")
                   for c in range(CT)]
            kvt = [acts.tile([P, CAP], BF16, tag=f"kv{c}", name=f"kv{c}")
                   for c in range(CT)]

            # DMA issue order tracks first-use order (queues are FIFO):
            # xk tb0, first expert weight, xk tb1, second weight, xk rest.
            # xr is NOT issued here — it would head-of-line-block the
            # in-loop wek fetches; it goes out after the expert-L1 loop.
            for cc in range(CT):
                nc.sync.dma_start(xkt[cc][:, tbs[0]], xk_d[cc, :, tbs[0]])
            wek_tiles = {}
            wt = wfp.tile([P, CT * P], BF16, tag="w1")
            _dma_chunked(nc, wt, wek[0], CT * P, 512)
            wek_tiles[0] = wt
            for cc in range(CT):
                nc.sync.dma_start(xkt[cc][:, tbs[1]], xk_d[cc, :, tbs[1]])
            wt = wfp.tile([P, CT * P], BF16, tag="w1")
            _dma_chunked(nc, wt, wek[1], CT * P, 512)
            wek_tiles[1] = wt
            for t in range(2, NT):
                for cc in range(CT):
                    nc.sync.dma_start(xkt[cc][:, tbs[t]], xk_d[cc, :, tbs[t]])
            tmask = constp.tile([P, CAP], BF16)
            nc.sync.dma_start(tmask[:], maskd[:])

            def l1_phase(wdram, m0, mts, xtiles, pre):
                """hidden[i] = relu(x @ W[m0+i])^2 for i in range(mts), bf16."""
                out = []
                for i in range(mts):
                    m = m0 + i
                    if m in pre:
                        wt = pre[m]
                    else:
                        wt = wfp.tile([P, CT * P], BF16, tag="w1")
                        _dma_chunked(nc, wt, wdram[m], CT * P, 512)
                    ht = acts.tile([P, CAP], BF16, tag=f"h{i}", name=f"h{i}")
                    out.append(ht)
                    for t in range(NT):
                        pd = ps1.tile([P, TW], F32, tag="ps1")
                        for k in range(CT):
                            nc.tensor.matmul(
                                pd[:], wt[:, k * P:(k + 1) * P],
                                xtiles[k][:, tbs[t]],
                                start=(k == 0), stop=(k == CT - 1),
                            )
                        rl = tmpp.tile([P, TW], BF16, tag="rl")
                        nc.scalar.activation(
                            rl[:], pd[:], mybir.ActivationFunctionType.Relu
                        )
                        nc.vector.tensor_tensor(
                            out=ht[:, tbs[t]], in0=rl[:], in1=rl[:],
                            op=mybir.AluOpType.mult,
                        )
                return out

            def l2_chain(po, wt, htiles, t, nk):
                for k in range(nk):
                    nc.tensor.matmul(
                        po[:], wt[:, k * P:(k + 1) * P],
                        htiles[k][:, tbs[t]],
                        start=(k == 0), stop=(k == nk - 1),
                    )

            # ---- expert FFN first: kv = mask * (relu(xk@Wek)^2 @ Wev) ----
            ht = l1_phase(wek, 0, MT_E, xkt, wek_tiles)
            # xr is needed only by the receptance at the very end; issue
            # its DMAs here so they ride behind the expert-L1 weights.
            for t in range(NT):
                for cc in range(CT):
                    nc.sync.dma_start(xrt[cc][:, tbs[t]], xr_d[cc, :, tbs[t]])
            for c in range(CT):
                wt = wsp.tile([P, MT_E * P], BF16, tag="w2")
                _dma_chunked(nc, wt, wev[c], MT_E * P, 512)
                for t in range(NT):
                    po = ps2.tile([P, TW], F32, tag="ps2")
                    l2_chain(po, wt, ht, t, MT_E)
                    nc.vector.tensor_tensor(
                        out=kvt[c][:, tbs[t]], in0=po[:], in1=tmask[:, tbs[t]],
                        op=mybir.AluOpType.mult,
                    )

            # ---- dense FFN: kv += relu(xk@Wk)^2 @ Wv, 2 k-groups ----
            for g in range(GD):
                kt = l1_phase(wk, g * HD, HD, xkt, {})
                last_group = g == GD - 1
                for c in range(CT):
                    wt = wsp.tile([P, HD * P], BF16, tag="w2")
                    _dma_chunked(
                        nc, wt, wv[c, :, g * HD * P:(g + 1) * HD * P],
                        HD * P, 512,
                    )
                    for t in range(NT):
                        pv = ps2.tile([P, TW], F32, tag="ps2")
                        l2_chain(pv, wt, kt, t, HD)
                        nc.vector.tensor_tensor(
                            out=kvt[c][:, tbs[t]], in0=pv[:],
                            in1=kvt[c][:, tbs[t]],
                            op=mybir.AluOpType.add,
                        )
                    if last_group:
                        # ---- receptance per c-tile as soon as kv[c] final:
                        # y = sigmoid(xr @ Wr) * kv
                        wrt = wfp.tile([P, CT * P], BF16, tag="w1")
                        _dma_chunked(nc, wrt, wr[c], CT * P, 512)
                        for t in range(NT):
                            pr = psr.tile([P, TW], F32, tag="psr")
                            for k in range(CT):
                                nc.tensor.matmul(
                                    pr[:], wrt[:, k * P:(k + 1) * P],
                                    xrt[k][:, tbs[t]],
                                    start=(k == 0), stop=(k == CT - 1),
                                )
                            rm = tmpp.tile([P, TW], BF16, tag="rm")
                            nc.scalar.activation(
                                rm[:], pr[:],
                                mybir.ActivationFunctionType.Sigmoid,
                            )
                            yo = outp.tile([P, TW], BF16, tag="yo")
                            nc.vector.tensor_tensor(
                                out=yo[:], in0=kvt[c][:, tbs[t]], in1=rm[:],
                                op=mybir.AluOpType.mult,
                            )
                            nc.sync.dma_start(yout[c, :, tbs[t]], yo[:])

    nc.compile()
    return nc


def _routing(token_ids: np.ndarray):
    """Token -> (per-core global token list [E, CAP], per-core keep mask)."""
    tid = token_ids.reshape(N).astype(np.int64)
    eidx = (tid * HASH_PRIME) % E
    order = np.argsort(eidx, kind="stable")  # FIFO within expert
    counts = np.bincount(eidx, minlength=E)
    starts = np.zeros(E + 1, np.int64)
    np.cumsum(counts, out=starts[1:])

    token_lists = np.empty((E, CAP), np.int64)
    masks = np.zeros((E, CAP), np.float32)
    dropped = []
    fill_needed = []
    for e in range(E):
        grp = order[starts[e]:starts[e + 1]]
        nk = min(len(grp), CAP)
        token_lists[e, :nk] = grp[:nk]
        masks[e, :nk] = 1.0
        dropped.append(grp[CAP:])
        fill_needed.append(CAP - nk)
    dropped = (
        np.concatenate(dropped) if dropped else np.empty(0, np.int64)
    )
    pos = 0
    for e in range(E):
        need = fill_needed[e]
        if need:
            token_lists[e, CAP - need:] = dropped[pos:pos + need]
            pos += need
    assert pos == len(dropped)
    return token_lists, masks


def _tile_first(W, mt):
    """[C, M] -> [mt, P, CT*P] with w[m][p][k*P+q] = W[k*P+p, m*P+q]."""
    ct = W.shape[0] // P
    return np.ascontiguousarray(
        W.reshape(ct, P, mt, P).transpose(2, 1, 0, 3).reshape(mt, P, ct * P)
    )


def _tile_second(W, ct_out):
    """[K, M] -> [ct_out, P, KT*P] with w[m][p][k*P+q] = W[k*P+p, m*P+q]."""
    kt = W.shape[0] // P
    return np.ascontiguousarray(
        W.reshape(kt, P, ct_out, P).transpose(2, 1, 0, 3).reshape(ct_out, P, kt * P)
    )


def kernel(x, shift_state, token_ids, time_maa_k, time_maa_r, Wk, Wv, Wr, Wek, Wev):
    global _COMPILED
    if _COMPILED is None:
        _COMPILED = _build()
    nc = _COMPILED

    x = np.asarray(x, np.float32)
    shift_state = np.asarray(shift_state, np.float32)
    token_lists, masks = _routing(np.asarray(token_ids))

    xf = x.reshape(N, C)
    xprev_f = np.empty_like(xf)
    xprev_f[1:] = xf[:-1]
    xprev_f[np.arange(B) * T] = shift_state

    # token shift on host: xk/xr = x + (xprev - x) * maa
    dx = xprev_f - xf
    maak = np.asarray(time_maa_k, np.float32)
    maar = np.asarray(time_maa_r, np.float32)
    xk_full = xf + dx * maak
    xr_full = xf + dx * maar

    bf = ml_dtypes.bfloat16
    wk_t = _tile_first(np.asarray(Wk, np.float32), MT_D).astype(bf)
    wr_t = _tile_first(np.asarray(Wr, np.float32), CT).astype(bf)
    wv_t = _tile_second(np.asarray(Wv, np.float32), CT).astype(bf)
    Wek = np.asarray(Wek, np.float32)
    Wev = np.asarray(Wev, np.float32)

    def ctmajor(rows):  # [CAP, C] -> [CT, P, CAP] bf16
        return np.ascontiguousarray(rows.T.reshape(CT, P, CAP)).astype(bf)

    in_maps = []
    for e in range(E):
        L = token_lists[e]
        in_maps.append(dict(
            xk=ctmajor(xk_full[L]),
            xr=ctmajor(xr_full[L]),
            maskd=np.ascontiguousarray(
                np.broadcast_to(masks[e], (P, CAP))
            ).astype(bf),
            wk=wk_t,
            wv=wv_t,
            wr=wr_t,
            wek=_tile_first(Wek[e], MT_E).astype(bf),
            wev=_tile_second(Wev[e], CT).astype(bf),
        ))

    res = run_bass_kernel_spmd(
        nc, in_maps, core_ids=list(range(E)),
        trace=bool(os.environ.get("KERNEL_TRACE")),
    )
    global LAST_RESULTS
    LAST_RESULTS = res

    y = np.empty((N, C), np.float32)
    for e in range(E):
        y[token_lists[e]] = (
            res.results[e]["y"].reshape(C, CAP).T.astype(np.float32)
        )
    return y.reshape(B, T, C)


# revision 17
# speedup vs baseline: 1.0032x; 1.0001x over previous
"""Trainium2 Bass kernel for nn_CMix_x060moe (RWKV CMix + hash-routed MoE).

Strategy: expert-sharded SPMD over 8 NeuronCores. Hash routing depends only
on token_ids, so the host computes the token->expert assignment as part of
sharding: core e receives exactly 2048 tokens (expert e's kept tokens in
FIFO order, padded with capacity-dropped tokens from anywhere, mask=0 for
those). The RWKV token-shift (xk/xr = x + (xprev-x)*maa) is folded into the
host-side gather, so the device receives xk/xr directly and every device
instruction is matmul roofline work. Each core computes the dense
squared-ReLU FFN, its own expert's FFN and the sigmoid receptance for its
2048 tokens; the host scatters rows back. No collectives needed and the
load is perfectly balanced.

All activations live C-major ("transposed", [C, tokens]) on device so every
matmul keeps weights as the stationary operand. Weights and activations are
bf16 (full PE rate, LDWEIGHTS fully hidden, half the HBM traffic of f32);
PSUM accumulation stays f32. Weights are streamed exactly once: the token
block is processed in a single pass with the dense second-layer contraction
split into 2 groups so the hidden activations fit SBUF.
"""

import os

import ml_dtypes
import numpy as np

import concourse.mybir as mybir
import concourse.tile as tile
from concourse import bacc
from concourse.bass_utils import run_bass_kernel_spmd

LAST_RESULTS = None  # set on every kernel() call; holds BassKernelResults

B, T, C = 8, 2048, 1024
DFF, DFFE = 4096, 2048
E = 8
HASH_PRIME = 5099
CAP = (B * T) // E  # 2048 tokens per core
N = B * T

P = 128               # partitions
TW = 512              # matmul token width (one f32 psum bank)
NT = CAP // TW        # 4 token blocks
CT = C // P           # 8  C-tiles
MT_D = DFF // P       # 32 dense-hidden tiles
MT_E = DFFE // P      # 16 expert-hidden tiles
GD = 2                # dense second-layer contraction groups
HD = MT_D // GD       # 16 k-tiles per dense group

F32 = mybir.dt.float32
BF16 = mybir.dt.bfloat16

_COMPILED = None


def _dma_chunked(nc, dst, src, width, chunk):
    """Split a wide weight DMA into column chunks so each rides its own
    HWDGE queue (single-queue BW is ~1/16th of aggregate)."""
    for o in range(0, width, chunk):
        e = min(o + chunk, width)
        nc.sync.dma_start(dst[:, o:e], src[:, o:e])


def _build():
    nc = bacc.Bacc(trn_type="TRN2")

    xk_d = nc.dram_tensor("xk", [CT, P, CAP], BF16, kind="ExternalInput")
    xr_d = nc.dram_tensor("xr", [CT, P, CAP], BF16, kind="ExternalInput")
    maskd = nc.dram_tensor("maskd", [P, CAP], BF16, kind="ExternalInput")
    # weights, host-tiled p-major: w*[m][p][k*P+q] = W[k*P+p, m*P+q]
    wk = nc.dram_tensor("wk", [MT_D, P, CT * P], BF16, kind="ExternalInput")
    wv = nc.dram_tensor("wv", [CT, P, MT_D * P], BF16, kind="ExternalInput")
    wr = nc.dram_tensor("wr", [CT, P, CT * P], BF16, kind="ExternalInput")
    wek = nc.dram_tensor("wek", [MT_E, P, CT * P], BF16, kind="ExternalInput")
    wev = nc.dram_tensor("wev", [CT, P, MT_E * P], BF16, kind="ExternalInput")
    yout = nc.dram_tensor("y", [CT, P, CAP], BF16, kind="ExternalOutput")

    with tile.TileContext(nc) as tc:
        with (
            tc.tile_pool(name="const", bufs=1) as constp,
            tc.tile_pool(name="acts", bufs=1) as acts,
            tc.tile_pool(name="wfirst", bufs=3) as wfp,
            tc.tile_pool(name="wsecond", bufs=3) as wsp,
            tc.tile_pool(name="tmp", bufs=3) as tmpp,
            tc.tile_pool(name="outp", bufs=3) as outp,
            tc.tile_pool(name="ps1", bufs=3, space="PSUM") as ps1,
            tc.tile_pool(name="ps2", bufs=3, space="PSUM") as ps2,
            tc.tile_pool(name="psr", bufs=2, space="PSUM") as psr,
        ):
            tbs = [slice(t * TW, (t + 1) * TW) for t in range(NT)]

            # PE warm-up: the HAM clock gate holds the PE at 1.2 GHz until
            # ~3.4us of sustained activity. Run dummy matmuls on zeroed
            # tiles during the otherwise-idle input-DMA window so the real
            # chains start at 2.4 GHz. Sized to end just before the first
            # xk/wek data lands (~15us).
            dw = constp.tile([P, P], BF16, name="dummy_w")
            nc.vector.memzero(dw[:])
            da = constp.tile([P, TW], BF16, name="dummy_a")
            nc.vector.memzero(da[:])
            dps = ps1.tile([P, TW], F32, tag="ps1")
            NWARM = 28
            for i in range(NWARM):
                nc.tensor.matmul(dps[:], dw[:], da[:],
                                 start=(i == 0), stop=(i == NWARM - 1))

            # persistent activations (bf16): xk/xr inputs, kv accumulator
            xkt = [acts.tile([P, CAP], BF16, tag=f"xk{c}", name=f"xk{c}")
                   for c in range(CT)]
            xrt = [acts.tile([P, CAP], BF16, tag=f"xr{c}", name=f"xr{c}")
                   for c in range(CT)]
            kvt = [acts.tile([P, CAP], BF16, tag=f"kv{c}", name=f"kv{c}")
                   for c in range(CT)]

            # DMA issue order tracks first-use order (queues are FIFO):
            # xk tb0, first expert weight, xk tb1, second weight, xk rest.
            # xr is NOT issued here — it would head-of-line-block the
            # in-loop wek fetches; it goes out after the expert-L1 loop.
            for cc in range(CT):
                nc.sync.dma_start(xkt[cc][:, tbs[0]], xk_d[cc, :, tbs[0]])
            wek_tiles = {}
            wt = wfp.tile([P, CT * P], BF16, tag="w1")
            _dma_chunked(nc, wt, wek[0], CT * P, 512)
            wek_tiles[0] = wt
            for cc in range(CT):
                nc.sync.dma_start(xkt[cc][:, tbs[1]], xk_d[cc, :, tbs[1]])
            wt = wfp.tile([P, CT * P], BF16, tag="w1")
            _dma_chunked(nc, wt, wek[1], CT * P, 512)
            wek_tiles[1] = wt
            for t in range(2, NT):
                for cc in range(CT):
                    nc.sync.dma_start(xkt[cc][:, tbs[t]], xk_d[cc, :, tbs[t]])
            tmask = constp.tile([P, CAP], BF16)
            nc.sync.dma_start(tmask[:], maskd[:])

            def l1_phase(wdram, m0, mts, xtiles, pre):
                """hidden[i] = relu(x @ W[m0+i])^2 for i in range(mts), bf16."""
                out = []
                for i in range(mts):
                    m = m0 + i
                    if m in pre:
                        wt = pre[m]
                    else:
                        wt = wfp.tile([P, CT * P], BF16, tag="w1")
                        _dma_chunked(nc, wt, wdram[m], CT * P, 512)
                    ht = acts.tile([P, CAP], BF16, tag=f"h{i}", name=f"h{i}")
                    out.append(ht)
                    for t in range(NT):
                        pd = ps1.tile([P, TW], F32, tag="ps1")
                        for k in range(CT):
                            nc.tensor.matmul(
                                pd[:], wt[:, k * P:(k + 1) * P],
                                xtiles[k][:, tbs[t]],
                                start=(k == 0), stop=(k == CT - 1),
                            )
                        rl = tmpp.tile([P, TW], BF16, tag="rl")
                        nc.scalar.activation(
                            rl[:], pd[:], mybir.ActivationFunctionType.Relu
                        )
                        nc.vector.tensor_tensor(
                            out=ht[:, tbs[t]], in0=rl[:], in1=rl[:],
                            op=mybir.AluOpType.mult,
                        )
                return out

            def l2_chain(po, wt, htiles, t, nk):
                for k in range(nk):
                    nc.tensor.matmul(
                        po[:], wt[:, k * P:(k + 1) * P],
                        htiles[k][:, tbs[t]],
                        start=(k == 0), stop=(k == nk - 1),
                    )

            # ---- expert FFN first: kv = mask * (relu(xk@Wek)^2 @ Wev) ----
            ht = l1_phase(wek, 0, MT_E, xkt, wek_tiles)
            # xr is needed only by the receptance at the very end; issue
            # its DMAs here so they ride behind the expert-L1 weights.
            for t in range(NT):
                for cc in range(CT):
                    nc.sync.dma_start(xrt[cc][:, tbs[t]], xr_d[cc, :, tbs[t]])
            for c in range(CT):
                wt = wsp.tile([P, MT_E * P], BF16, tag="w2")
                _dma_chunked(nc, wt, wev[c], MT_E * P, 512)
                for t in range(NT):
                    po = ps2.tile([P, TW], F32, tag="ps2")
                    l2_chain(po, wt, ht, t, MT_E)
                    nc.vector.tensor_tensor(
                        out=kvt[c][:, tbs[t]], in0=po[:], in1=tmask[:, tbs[t]],
                        op=mybir.AluOpType.mult,
                    )

            # ---- dense FFN: kv += relu(xk@Wk)^2 @ Wv, 2 k-groups ----
            for g in range(GD):
                kt = l1_phase(wk, g * HD, HD, xkt, {})
                last_group = g == GD - 1
                for c in range(CT):
                    wt = wsp.tile([P, HD * P], BF16, tag="w2")
                    _dma_chunked(
                        nc, wt, wv[c, :, g * HD * P:(g + 1) * HD * P],
                        HD * P, 512,
                    )
                    for t in range(NT):
                        pv = ps2.tile([P, TW], F32, tag="ps2")
                        l2_chain(pv, wt, kt, t, HD)
                        nc.vector.tensor_tensor(
                            out=kvt[c][:, tbs[t]], in0=pv[:],
                            in1=kvt[c][:, tbs[t]],
                            op=mybir.AluOpType.add,
                        )
                    if last_group:
                        # ---- receptance per c-tile as soon as kv[c] final:
                        # y = sigmoid(xr @ Wr) * kv
                        wrt = wfp.tile([P, CT * P], BF16, tag="w1")
                        _dma_chunked(nc, wrt, wr[c], CT * P, 512)
                        for t in range(NT):
                            pr = psr.tile([P, TW], F32, tag="psr")
                            for k in range(CT):
                                nc.tensor.matmul(
                                    pr[:], wrt[:, k * P:(k + 1) * P],
                                    xrt[k][:, tbs[t]],
                                    start=(k == 0), stop=(k == CT - 1),
                                )
                            rm = tmpp.tile([P, TW], BF16, tag="rm")
                            nc.scalar.activation(
                                rm[:], pr[:],
                                mybir.ActivationFunctionType.Sigmoid,
                            )
                            yo = outp.tile([P, TW], BF16, tag="yo")
                            nc.vector.tensor_tensor(
                                out=yo[:], in0=kvt[c][:, tbs[t]], in1=rm[:],
                                op=mybir.AluOpType.mult,
                            )
                            # 2 chunks so the tail-critical last store
                            # rides two queues
                            h = TW // 2
                            nc.sync.dma_start(
                                yout[c, :, t * TW:t * TW + h], yo[:, :h]
                            )
                            nc.sync.dma_start(
                                yout[c, :, t * TW + h:(t + 1) * TW], yo[:, h:]
                            )

    nc.compile()
    return nc


def _routing(token_ids: np.ndarray):
    """Token -> (per-core global token list [E, CAP], per-core keep mask)."""
    tid = token_ids.reshape(N).astype(np.int64)
    eidx = (tid * HASH_PRIME) % E
    order = np.argsort(eidx, kind="stable")  # FIFO within expert
    counts = np.bincount(eidx, minlength=E)
    starts = np.zeros(E + 1, np.int64)
    np.cumsum(counts, out=starts[1:])

    token_lists = np.empty((E, CAP), np.int64)
    masks = np.zeros((E, CAP), np.float32)
    dropped = []
    fill_needed = []
    for e in range(E):
        grp = order[starts[e]:starts[e + 1]]
        nk = min(len(grp), CAP)
        token_lists[e, :nk] = grp[:nk]
        masks[e, :nk] = 1.0
        dropped.append(grp[CAP:])
        fill_needed.append(CAP - nk)
    dropped = (
        np.concatenate(dropped) if dropped else np.empty(0, np.int64)
    )
    pos = 0
    for e in range(E):
        need = fill_needed[e]
        if need:
            token_lists[e, CAP - need:] = dropped[pos:pos + need]
            pos += need
    assert pos == len(dropped)
    return token_lists, masks


def _tile_first(W, mt):
    """[C, M] -> [mt, P, CT*P] with w[m][p][k*P+q] = W[k*P+p, m*P+q]."""
    ct = W.shape[0] // P
    return np.ascontiguousarray(
        W.reshape(ct, P, mt, P).transpose(2, 1, 0, 3).reshape(mt, P, ct * P)
    )


def _tile_second(W, ct_out):
    """[K, M] -> [ct_out, P, KT*P] with w[m][p][k*P+q] = W[k*P+p, m*P+q]."""
    kt = W.shape[0] // P
    return np.ascontiguousarray(
        W.reshape(kt, P, ct_out, P).transpose(2, 1, 0, 3).reshape(ct_out, P, kt * P)
    )


def kernel(x, shift_state, token_ids, time_maa_k, time_maa_r, Wk, Wv, Wr, Wek, Wev):
    global _COMPILED
    if _COMPILED is None:
        _COMPILED = _build()
    nc = _COMPILED

    x = np.asarray(x, np.float32)
    shift_state = np.asarray(shift_state, np.float32)
    token_lists, masks = _routing(np.asarray(token_ids))

    xf = x.reshape(N, C)
    xprev_f = np.empty_like(xf)
    xprev_f[1:] = xf[:-1]
    xprev_f[np.arange(B) * T] = shift_state

    # token shift on host: xk/xr = x + (xprev - x) * maa
    dx = xprev_f - xf
    maak = np.asarray(time_maa_k, np.float32)
    maar = np.asarray(time_maa_r, np.float32)
    xk_full = xf + dx * maak
    xr_full = xf + dx * maar

    bf = ml_dtypes.bfloat16
    wk_t = _tile_first(np.asarray(Wk, np.float32), MT_D).astype(bf)
    wr_t = _tile_first(np.asarray(Wr, np.float32), CT).astype(bf)
    wv_t = _tile_second(np.asarray(Wv, np.float32), CT).astype(bf)
    Wek = np.asarray(Wek, np.float32)
    Wev = np.asarray(Wev, np.float32)

    def ctmajor(rows):  # [CAP, C] -> [CT, P, CAP] bf16
        return np.ascontiguousarray(rows.T.reshape(CT, P, CAP)).astype(bf)

    in_maps = []
    for e in range(E):
        L = token_lists[e]
        in_maps.append(dict(
            xk=ctmajor(xk_full[L]),
            xr=ctmajor(xr_full[L]),
            maskd=np.ascontiguousarray(
                np.broadcast_to(masks[e], (P, CAP))
            ).astype(bf),
            wk=wk_t,
            wv=wv_t,
            wr=wr_t,
            wek=_tile_first(Wek[e], MT_E).astype(bf),
            wev=_tile_second(Wev[e], CT).astype(bf),
        ))

    res = run_bass_kernel_spmd(
        nc, in_maps, core_ids=list(range(E)),
        trace=bool(os.environ.get("KERNEL_TRACE")),
    )
    global LAST_RESULTS
    LAST_RESULTS = res

    y = np.empty((N, C), np.float32)
    for e in range(E):
        y[token_lists[e]] = (
            res.results[e]["y"].reshape(C, CAP).T.astype(np.float32)
        )
    return y.reshape(B, T, C)


# revision 21
# speedup vs baseline: 1.0036x; 1.0004x over previous
"""Trainium2 Bass kernel for nn_CMix_x060moe (RWKV CMix + hash-routed MoE).

Strategy: expert-sharded SPMD over 8 NeuronCores. Hash routing depends only
on token_ids, so the host computes the token->expert assignment as part of
sharding: core e receives exactly 2048 tokens (expert e's kept tokens in
FIFO order, padded with capacity-dropped tokens from anywhere, mask=0 for
those). The RWKV token-shift (xk/xr = x + (xprev-x)*maa) is folded into the
host-side gather, so the device receives xk/xr directly and every device
instruction is matmul roofline work. Each core computes the dense
squared-ReLU FFN, its own expert's FFN and the sigmoid receptance for its
2048 tokens; the host scatters rows back. No collectives needed and the
load is perfectly balanced.

All activations live C-major ("transposed", [C, tokens]) on device so every
matmul keeps weights as the stationary operand. Weights and activations are
bf16 (full PE rate, LDWEIGHTS fully hidden, half the HBM traffic of f32);
PSUM accumulation stays f32. Weights are streamed exactly once: the token
block is processed in a single pass with the dense second-layer contraction
split into 2 groups so the hidden activations fit SBUF.
"""

import os

import ml_dtypes
import numpy as np

import concourse.mybir as mybir
import concourse.tile as tile
from concourse import bacc
from concourse.bass_utils import run_bass_kernel_spmd

LAST_RESULTS = None  # set on every kernel() call; holds BassKernelResults

B, T, C = 8, 2048, 1024
DFF, DFFE = 4096, 2048
E = 8
HASH_PRIME = 5099
CAP = (B * T) // E  # 2048 tokens per core
N = B * T

P = 128               # partitions
TW = 512              # matmul token width (one f32 psum bank)
NT = CAP // TW        # 4 token blocks
CT = C // P           # 8  C-tiles
MT_D = DFF // P       # 32 dense-hidden tiles
MT_E = DFFE // P      # 16 expert-hidden tiles
GD = 2                # dense second-layer contraction groups
HD = MT_D // GD       # 16 k-tiles per dense group

F32 = mybir.dt.float32
BF16 = mybir.dt.bfloat16

_COMPILED = None


def _dma_chunked(nc, dst, src, width, chunk):
    """Split a wide weight DMA into column chunks so each rides its own
    HWDGE queue (single-queue BW is ~1/16th of aggregate)."""
    for o in range(0, width, chunk):
        e = min(o + chunk, width)
        nc.sync.dma_start(dst[:, o:e], src[:, o:e])


def _build():
    nc = bacc.Bacc(trn_type="TRN2")

    xk_d = nc.dram_tensor("xk", [CT, P, CAP], BF16, kind="ExternalInput")
    xr_d = nc.dram_tensor("xr", [CT, P, CAP], BF16, kind="ExternalInput")
    maskd = nc.dram_tensor("maskd", [P, CAP], BF16, kind="ExternalInput")
    # weights, host-tiled p-major: w*[m][p][k*P+q] = W[k*P+p, m*P+q]
    wk = nc.dram_tensor("wk", [MT_D, P, CT * P], BF16, kind="ExternalInput")
    wv = nc.dram_tensor("wv", [CT, P, MT_D * P], BF16, kind="ExternalInput")
    wr = nc.dram_tensor("wr", [CT, P, CT * P], BF16, kind="ExternalInput")
    wek = nc.dram_tensor("wek", [MT_E, P, CT * P], BF16, kind="ExternalInput")
    wev = nc.dram_tensor("wev", [CT, P, MT_E * P], BF16, kind="ExternalInput")
    yout = nc.dram_tensor("y", [CT, P, CAP], BF16, kind="ExternalOutput")

    with tile.TileContext(nc) as tc:
        with (
            tc.tile_pool(name="const", bufs=1) as constp,
            tc.tile_pool(name="acts", bufs=1) as acts,
            tc.tile_pool(name="wfirst", bufs=3) as wfp,
            tc.tile_pool(name="wsecond", bufs=3) as wsp,
            tc.tile_pool(name="tmp", bufs=3) as tmpp,
            tc.tile_pool(name="outp", bufs=3) as outp,
            tc.tile_pool(name="ps1", bufs=3, space="PSUM") as ps1,
            tc.tile_pool(name="ps2", bufs=3, space="PSUM") as ps2,
            tc.tile_pool(name="psr", bufs=2, space="PSUM") as psr,
        ):
            tbs = [slice(t * TW, (t + 1) * TW) for t in range(NT)]

            # PE warm-up: the HAM clock gate holds the PE at 1.2 GHz until
            # ~3.4us of sustained activity. Run dummy matmuls on zeroed
            # tiles during the otherwise-idle input-DMA window so the real
            # chains start at 2.4 GHz. Sized to end just before the first
            # xk/wek data lands (~15us).
            dw = constp.tile([P, P], BF16, name="dummy_w")
            nc.vector.memzero(dw[:])
            da = constp.tile([P, TW], BF16, name="dummy_a")
            nc.vector.memzero(da[:])
            dps = ps1.tile([P, TW], F32, tag="ps1")
            NWARM = 28
            for i in range(NWARM):
                nc.tensor.matmul(dps[:], dw[:], da[:],
                                 start=(i == 0), stop=(i == NWARM - 1))

            # persistent activations (bf16): xk/xr inputs, kv accumulator
            xkt = [acts.tile([P, CAP], BF16, tag=f"xk{c}", name=f"xk{c}")
                   for c in range(CT)]
            xrt = [acts.tile([P, CAP], BF16, tag=f"xr{c}", name=f"xr{c}")
                   for c in range(CT)]
            kvt = [acts.tile([P, CAP], BF16, tag=f"kv{c}", name=f"kv{c}")
                   for c in range(CT)]

            # DMA issue order tracks first-use order (queues are FIFO):
            # xk tb0, first expert weight, xk tb1, second weight, xk rest.
            # xr is NOT issued here — it would head-of-line-block the
            # in-loop wek fetches; it goes out after the expert-L1 loop.
            for cc in range(CT):
                nc.sync.dma_start(xkt[cc][:, tbs[0]], xk_d[cc, :, tbs[0]])
            wek_tiles = {}
            wt = wfp.tile([P, CT * P], BF16, tag="w1")
            _dma_chunked(nc, wt, wek[0], CT * P, 512)
            wek_tiles[0] = wt
            for cc in range(CT):
                nc.sync.dma_start(xkt[cc][:, tbs[1]], xk_d[cc, :, tbs[1]])
            wt = wfp.tile([P, CT * P], BF16, tag="w1")
            _dma_chunked(nc, wt, wek[1], CT * P, 512)
            wek_tiles[1] = wt
            for t in range(2, NT):
                for cc in range(CT):
                    nc.sync.dma_start(xkt[cc][:, tbs[t]], xk_d[cc, :, tbs[t]])
            tmask = constp.tile([P, CAP], BF16)
            nc.sync.dma_start(tmask[:], maskd[:])

            def l1_phase(wdram, m0, mts, xtiles, pre, ramp=False):
                """hidden[i] = relu(x @ W[m0+i])^2 for i in range(mts), bf16.

                ramp=True (first phase only): the first two m-tiles run
                their t0/t1 chains before t2/t3 so the PE only needs half
                of xk during the DMA-bound startup window."""
                if ramp:
                    order = [(0, 0), (0, 1), (1, 0), (1, 1),
                             (0, 2), (0, 3), (1, 2), (1, 3)]
                    order += [(i, t) for i in range(2, mts) for t in range(NT)]
                else:
                    order = [(i, t) for i in range(mts) for t in range(NT)]
                wts, hts = {}, {}
                for i, t in order:
                    if i not in wts:
                        m = m0 + i
                        if m in pre:
                            wts[i] = pre[m]
                        else:
                            wts[i] = wfp.tile([P, CT * P], BF16, tag="w1",
                                              name=f"w1_{m}")
                            _dma_chunked(nc, wts[i], wdram[m], CT * P, 512)
                        hts[i] = acts.tile([P, CAP], BF16, tag=f"h{i}",
                                           name=f"h{i}")
                    wt, ht = wts[i], hts[i]
                    pd = ps1.tile([P, TW], F32, tag="ps1")
                    for k in range(CT):
                        nc.tensor.matmul(
                            pd[:], wt[:, k * P:(k + 1) * P],
                            xtiles[k][:, tbs[t]],
                            start=(k == 0), stop=(k == CT - 1),
                        )
                    rl = tmpp.tile([P, TW], BF16, tag="rl")
                    nc.scalar.activation(
                        rl[:], pd[:], mybir.ActivationFunctionType.Relu
                    )
                    nc.vector.tensor_tensor(
                        out=ht[:, tbs[t]], in0=rl[:], in1=rl[:],
                        op=mybir.AluOpType.mult,
                    )
                return [hts[i] for i in range(mts)]

            def l2_chain(po, wt, htiles, t, nk):
                for k in range(nk):
                    nc.tensor.matmul(
                        po[:], wt[:, k * P:(k + 1) * P],
                        htiles[k][:, tbs[t]],
                        start=(k == 0), stop=(k == nk - 1),
                    )

            # ---- expert FFN first: kv = mask * (relu(xk@Wek)^2 @ Wev) ----
            ht = l1_phase(wek, 0, MT_E, xkt, wek_tiles, ramp=True)
            # xr is needed only by the receptance at the very end; issue
            # its DMAs here so they ride behind the expert-L1 weights.
            for t in range(NT):
                for cc in range(CT):
                    nc.sync.dma_start(xrt[cc][:, tbs[t]], xr_d[cc, :, tbs[t]])
            for c in range(CT):
                wt = wsp.tile([P, MT_E * P], BF16, tag="w2")
                _dma_chunked(nc, wt, wev[c], MT_E * P, 512)
                for t in range(NT):
                    po = ps2.tile([P, TW], F32, tag="ps2")
                    l2_chain(po, wt, ht, t, MT_E)
                    nc.vector.tensor_tensor(
                        out=kvt[c][:, tbs[t]], in0=po[:], in1=tmask[:, tbs[t]],
                        op=mybir.AluOpType.mult,
                    )

            # ---- dense FFN: kv += relu(xk@Wk)^2 @ Wv, 2 k-groups ----
            for g in range(GD):
                kt = l1_phase(wk, g * HD, HD, xkt, {})
                last_group = g == GD - 1
                for c in range(CT):
                    wt = wsp.tile([P, HD * P], BF16, tag="w2")
                    _dma_chunked(
                        nc, wt, wv[c, :, g * HD * P:(g + 1) * HD * P],
                        HD * P, 512,
                    )
                    for t in range(NT):
                        pv = ps2.tile([P, TW], F32, tag="ps2")
                        l2_chain(pv, wt, kt, t, HD)
                        nc.vector.tensor_tensor(
                            out=kvt[c][:, tbs[t]], in0=pv[:],
                            in1=kvt[c][:, tbs[t]],
                            op=mybir.AluOpType.add,
                        )
                    if last_group:
                        # ---- receptance per c-tile as soon as kv[c] final:
                        # y = sigmoid(xr @ Wr) * kv. Chains run 256 wide
                        # (each in its own rotating psum bank) so the
                        # sigmoid/mult/store pipeline after the very last
                        # matmul handles 4x less data.
                        TR = TW // 2
                        wrt = wfp.tile([P, CT * P], BF16, tag="w1")
                        _dma_chunked(nc, wrt, wr[c], CT * P, 512)
                        for s in range(CAP // TR):
                            sub = slice(s * TR, (s + 1) * TR)
                            pr = psr.tile([P, TW], F32, tag="psr")
                            for k in range(CT):
                                nc.tensor.matmul(
                                    pr[:, :TR], wrt[:, k * P:(k + 1) * P],
                                    xrt[k][:, sub],
                                    start=(k == 0), stop=(k == CT - 1),
                                )
                            rm = tmpp.tile([P, TR], BF16, tag="rm")
                            nc.scalar.activation(
                                rm[:], pr[:, :TR],
                                mybir.ActivationFunctionType.Sigmoid,
                            )
                            yo = outp.tile([P, TR], BF16, tag="yo")
                            nc.vector.tensor_tensor(
                                out=yo[:], in0=kvt[c][:, sub], in1=rm[:],
                                op=mybir.AluOpType.mult,
                            )
                            # 2 chunks so the tail-critical last store
                            # rides two queues
                            h = TR // 2
                            nc.sync.dma_start(
                                yout[c, :, s * TR:s * TR + h], yo[:, :h]
                            )
                            nc.sync.dma_start(
                                yout[c, :, s * TR + h:(s + 1) * TR], yo[:, h:]
                            )

    nc.compile()
    return nc


def _routing(token_ids: np.ndarray):
    """Token -> (per-core global token list [E, CAP], per-core keep mask)."""
    tid = token_ids.reshape(N).astype(np.int64)
    eidx = (tid * HASH_PRIME) % E
    order = np.argsort(eidx, kind="stable")  # FIFO within expert
    counts = np.bincount(eidx, minlength=E)
    starts = np.zeros(E + 1, np.int64)
    np.cumsum(counts, out=starts[1:])

    token_lists = np.empty((E, CAP), np.int64)
    masks = np.zeros((E, CAP), np.float32)
    dropped = []
    fill_needed = []
    for e in range(E):
        grp = order[starts[e]:starts[e + 1]]
        nk = min(len(grp), CAP)
        token_lists[e, :nk] = grp[:nk]
        masks[e, :nk] = 1.0
        dropped.append(grp[CAP:])
        fill_needed.append(CAP - nk)
    dropped = (
        np.concatenate(dropped) if dropped else np.empty(0, np.int64)
    )
    pos = 0
    for e in range(E):
        need = fill_needed[e]
        if need:
            token_lists[e, CAP - need:] = dropped[pos:pos + need]
            pos += need
    assert pos == len(dropped)
    return token_lists, masks


def _tile_first(W, mt):
    """[C, M] -> [mt, P, CT*P] with w[m][p][k*P+q] = W[k*P+p, m*P+q]."""
    ct = W.shape[0] // P
    return np.ascontiguousarray(
        W.reshape(ct, P, mt, P).transpose(2, 1, 0, 3).reshape(mt, P, ct * P)
    )


def _tile_second(W, ct_out):
    """[K, M] -> [ct_out, P, KT*P] with w[m][p][k*P+q] = W[k*P+p, m*P+q]."""
    kt = W.shape[0] // P
    return np.ascontiguousarray(
        W.reshape(kt, P, ct_out, P).transpose(2, 1, 0, 3).reshape(ct_out, P, kt * P)
    )


def kernel(x, shift_state, token_ids, time_maa_k, time_maa_r, Wk, Wv, Wr, Wek, Wev):
    global _COMPILED
    if _COMPILED is None:
        _COMPILED = _build()
    nc = _COMPILED

    x = np.asarray(x, np.float32)
    shift_state = np.asarray(shift_state, np.float32)
    token_lists, masks = _routing(np.asarray(token_ids))

    xf = x.reshape(N, C)
    xprev_f = np.empty_like(xf)
    xprev_f[1:] = xf[:-1]
    xprev_f[np.arange(B) * T] = shift_state

    # token shift on host: xk/xr = x + (xprev - x) * maa
    dx = xprev_f - xf
    maak = np.asarray(time_maa_k, np.float32)
    maar = np.asarray(time_maa_r, np.float32)
    xk_full = xf + dx * maak
    xr_full = xf + dx * maar

    bf = ml_dtypes.bfloat16
    wk_t = _tile_first(np.asarray(Wk, np.float32), MT_D).astype(bf)
    wr_t = _tile_first(np.asarray(Wr, np.float32), CT).astype(bf)
    wv_t = _tile_second(np.asarray(Wv, np.float32), CT).astype(bf)
    Wek = np.asarray(Wek, np.float32)
    Wev = np.asarray(Wev, np.float32)

    def ctmajor(rows):  # [CAP, C] -> [CT, P, CAP] bf16
        return np.ascontiguousarray(rows.T.reshape(CT, P, CAP)).astype(bf)

    in_maps = []
    for e in range(E):
        L = token_lists[e]
        in_maps.append(dict(
            xk=ctmajor(xk_full[L]),
            xr=ctmajor(xr_full[L]),
            maskd=np.ascontiguousarray(
                np.broadcast_to(masks[e], (P, CAP))
            ).astype(bf),
            wk=wk_t,
            wv=wv_t,
            wr=wr_t,
            wek=_tile_first(Wek[e], MT_E).astype(bf),
            wev=_tile_second(Wev[e], CT).astype(bf),
        ))

    res = run_bass_kernel_spmd(
        nc, in_maps, core_ids=list(range(E)),
        trace=bool(os.environ.get("KERNEL_TRACE")),
    )
    global LAST_RESULTS
    LAST_RESULTS = res

    y = np.empty((N, C), np.float32)
    for e in range(E):
        y[token_lists[e]] = (
            res.results[e]["y"].reshape(C, CAP).T.astype(np.float32)
        )
    return y.reshape(B, T, C)


# revision 22
# speedup vs baseline: 1.0094x; 1.0057x over previous
"""Trainium2 Bass kernel for nn_CMix_x060moe (RWKV CMix + hash-routed MoE).

Strategy: expert-sharded SPMD over 8 NeuronCores. Hash routing depends only
on token_ids, so the host computes the token->expert assignment as part of
sharding: core e receives exactly 2048 tokens (expert e's kept tokens in
FIFO order, padded with capacity-dropped tokens from anywhere, mask=0 for
those). The RWKV token-shift (xk/xr = x + (xprev-x)*maa) is folded into the
host-side gather, so the device receives xk/xr directly and every device
instruction is matmul roofline work. Each core computes the dense
squared-ReLU FFN, its own expert's FFN and the sigmoid receptance for its
2048 tokens; the host scatters rows back. No collectives needed and the
load is perfectly balanced.

All activations live C-major ("transposed", [C, tokens]) on device so every
matmul keeps weights as the stationary operand. Weights and activations are
bf16 (full PE rate, LDWEIGHTS fully hidden, half the HBM traffic of f32);
PSUM accumulation stays f32. Weights are streamed exactly once: the token
block is processed in a single pass with the dense second-layer contraction
split into 2 groups so the hidden activations fit SBUF.
"""

import os

import ml_dtypes
import numpy as np

import concourse.mybir as mybir
import concourse.tile as tile
from concourse import bacc
from concourse.bass_utils import run_bass_kernel_spmd

LAST_RESULTS = None  # set on every kernel() call; holds BassKernelResults

B, T, C = 8, 2048, 1024
DFF, DFFE = 4096, 2048
E = 8
HASH_PRIME = 5099
CAP = (B * T) // E  # 2048 tokens per core
N = B * T

P = 128               # partitions
TW = 512              # matmul token width (one f32 psum bank)
NT = CAP // TW        # 4 token blocks
CT = C // P           # 8  C-tiles
MT_D = DFF // P       # 32 dense-hidden tiles
MT_E = DFFE // P      # 16 expert-hidden tiles
GD = 2                # dense second-layer contraction groups
HD = MT_D // GD       # 16 k-tiles per dense group

F32 = mybir.dt.float32
BF16 = mybir.dt.bfloat16

_COMPILED = None


def _dma_chunked(nc, dst, src, width, chunk):
    """Split a wide weight DMA into column chunks so each rides its own
    HWDGE queue (single-queue BW is ~1/16th of aggregate)."""
    for o in range(0, width, chunk):
        e = min(o + chunk, width)
        nc.sync.dma_start(dst[:, o:e], src[:, o:e])


def _build():
    nc = bacc.Bacc(trn_type="TRN2")

    xk_d = nc.dram_tensor("xk", [CT, P, CAP], BF16, kind="ExternalInput")
    xr_d = nc.dram_tensor("xr", [CT, P, CAP], BF16, kind="ExternalInput")
    maskd = nc.dram_tensor("maskd", [P, CAP], BF16, kind="ExternalInput")
    # weights, host-tiled p-major: w*[m][p][k*P+q] = W[k*P+p, m*P+q]
    wk = nc.dram_tensor("wk", [MT_D, P, CT * P], BF16, kind="ExternalInput")
    wv = nc.dram_tensor("wv", [CT, P, MT_D * P], BF16, kind="ExternalInput")
    wr = nc.dram_tensor("wr", [CT, P, CT * P], BF16, kind="ExternalInput")
    wek = nc.dram_tensor("wek", [MT_E, P, CT * P], BF16, kind="ExternalInput")
    wev = nc.dram_tensor("wev", [CT, P, MT_E * P], BF16, kind="ExternalInput")
    yout = nc.dram_tensor("y", [CT, P, CAP], BF16, kind="ExternalOutput")

    with tile.TileContext(nc) as tc:
        with (
            tc.tile_pool(name="const", bufs=1) as constp,
            tc.tile_pool(name="acts", bufs=1) as acts,
            tc.tile_pool(name="wfirst", bufs=3) as wfp,
            tc.tile_pool(name="wsecond", bufs=3) as wsp,
            tc.tile_pool(name="tmp", bufs=3) as tmpp,
            tc.tile_pool(name="outp", bufs=3) as outp,
            tc.tile_pool(name="ps1", bufs=3, space="PSUM") as ps1,
            tc.tile_pool(name="ps2", bufs=3, space="PSUM") as ps2,
            tc.tile_pool(name="psr", bufs=2, space="PSUM") as psr,
        ):
            tbs = [slice(t * TW, (t + 1) * TW) for t in range(NT)]

            # PE warm-up: the HAM clock gate holds the PE at 1.2 GHz until
            # ~3.4us of sustained activity. Run dummy matmuls on zeroed
            # tiles during the otherwise-idle input-DMA window so the real
            # chains start at 2.4 GHz. Sized to end just before the first
            # xk/wek data lands (~15us).
            dw = constp.tile([P, P], BF16, name="dummy_w")
            nc.vector.memzero(dw[:])
            da = constp.tile([P, TW], BF16, name="dummy_a")
            nc.vector.memzero(da[:])
            dps = ps1.tile([P, TW], F32, tag="ps1")
            NWARM = 28
            for i in range(NWARM):
                nc.tensor.matmul(dps[:], dw[:], da[:],
                                 start=(i == 0), stop=(i == NWARM - 1))

            # persistent activations (bf16): xk/xr inputs, kv accumulator
            xkt = [acts.tile([P, CAP], BF16, tag=f"xk{c}", name=f"xk{c}")
                   for c in range(CT)]
            xrt = [acts.tile([P, CAP], BF16, tag=f"xr{c}", name=f"xr{c}")
                   for c in range(CT)]
            kvt = [acts.tile([P, CAP], BF16, tag=f"kv{c}", name=f"kv{c}")
                   for c in range(CT)]

            # DMA issue order tracks first-use order (queues are FIFO):
            # xk tb0, first expert weight, xk tb1, second weight, xk rest.
            # xr is NOT issued here — it would head-of-line-block the
            # in-loop wek fetches; it goes out after the expert-L1 loop.
            for cc in range(CT):
                nc.sync.dma_start(xkt[cc][:, tbs[0]], xk_d[cc, :, tbs[0]])
            wek_tiles = {}
            wt = wfp.tile([P, CT * P], BF16, tag="w1")
            _dma_chunked(nc, wt, wek[0], CT * P, 512)
            wek_tiles[0] = wt
            for cc in range(CT):
                nc.sync.dma_start(xkt[cc][:, tbs[1]], xk_d[cc, :, tbs[1]])
            wt = wfp.tile([P, CT * P], BF16, tag="w1")
            _dma_chunked(nc, wt, wek[1], CT * P, 512)
            wek_tiles[1] = wt
            for t in range(2, NT):
                for cc in range(CT):
                    nc.sync.dma_start(xkt[cc][:, tbs[t]], xk_d[cc, :, tbs[t]])
            tmask = constp.tile([P, CAP], BF16)
            nc.sync.dma_start(tmask[:], maskd[:])

            def l1_phase(wdram, m0, mts, xtiles, pre, ramp=False):
                """hidden[i] = relu(x @ W[m0+i])^2 for i in range(mts), bf16.

                ramp=True (first phase only): the first two m-tiles run
                their t0/t1 chains before t2/t3 so the PE only needs half
                of xk during the DMA-bound startup window."""
                if ramp:
                    order = [(0, 0), (0, 1), (1, 0), (1, 1),
                             (0, 2), (0, 3), (1, 2), (1, 3)]
                    order += [(i, t) for i in range(2, mts) for t in range(NT)]
                else:
                    order = [(i, t) for i in range(mts) for t in range(NT)]
                wts, hts = {}, {}
                for i, t in order:
                    if i not in wts:
                        m = m0 + i
                        if m in pre:
                            wts[i] = pre[m]
                        else:
                            wts[i] = wfp.tile([P, CT * P], BF16, tag="w1",
                                              name=f"w1_{m}")
                            _dma_chunked(nc, wts[i], wdram[m], CT * P, 512)
                        hts[i] = acts.tile([P, CAP], BF16, tag=f"h{i}",
                                           name=f"h{i}")
                    wt, ht = wts[i], hts[i]
                    pd = ps1.tile([P, TW], F32, tag="ps1")
                    for k in range(CT):
                        nc.tensor.matmul(
                            pd[:], wt[:, k * P:(k + 1) * P],
                            xtiles[k][:, tbs[t]],
                            start=(k == 0), stop=(k == CT - 1),
                        )
                    rl = tmpp.tile([P, TW], BF16, tag="rl")
                    nc.scalar.activation(
                        rl[:], pd[:], mybir.ActivationFunctionType.Relu
                    )
                    nc.vector.tensor_tensor(
                        out=ht[:, tbs[t]], in0=rl[:], in1=rl[:],
                        op=mybir.AluOpType.mult,
                    )
                return [hts[i] for i in range(mts)]

            def l2_chain(po, wt, htiles, t, nk):
                for k in range(nk):
                    nc.tensor.matmul(
                        po[:], wt[:, k * P:(k + 1) * P],
                        htiles[k][:, tbs[t]],
                        start=(k == 0), stop=(k == nk - 1),
                    )

            # ---- expert FFN first: kv = mask * (relu(xk@Wek)^2 @ Wev) ----
            ht = l1_phase(wek, 0, MT_E, xkt, wek_tiles, ramp=True)
            # xr is needed only by the receptance at the very end; issue
            # its DMAs here so they ride behind the expert-L1 weights.
            for t in range(NT):
                for cc in range(CT):
                    nc.sync.dma_start(xrt[cc][:, tbs[t]], xr_d[cc, :, tbs[t]])
            for c in range(CT):
                wt = wsp.tile([P, MT_E * P], BF16, tag="w2")
                _dma_chunked(nc, wt, wev[c], MT_E * P, 512)
                for t in range(NT):
                    po = ps2.tile([P, TW], F32, tag="ps2")
                    l2_chain(po, wt, ht, t, MT_E)
                    nc.vector.tensor_tensor(
                        out=kvt[c][:, tbs[t]], in0=po[:], in1=tmask[:, tbs[t]],
                        op=mybir.AluOpType.mult,
                    )

            # ---- dense FFN: kv += relu(xk@Wk)^2 @ Wv, 2 k-groups ----
            for g in range(GD):
                kt = l1_phase(wk, g * HD, HD, xkt, {})
                last_group = g == GD - 1
                for c in range(CT):
                    wt = wsp.tile([P, HD * P], BF16, tag="w2")
                    _dma_chunked(
                        nc, wt, wv[c, :, g * HD * P:(g + 1) * HD * P],
                        HD * P, 512,
                    )
                    for t in range(NT):
                        pv = ps2.tile([P, TW], F32, tag="ps2")
                        l2_chain(pv, wt, kt, t, HD)
                        nc.vector.tensor_tensor(
                            out=kvt[c][:, tbs[t]], in0=pv[:],
                            in1=kvt[c][:, tbs[t]],
                            op=mybir.AluOpType.add,
                        )
                    if last_group:
                        # ---- receptance per c-tile as soon as kv[c] final:
                        # y = sigmoid(xr @ Wr) * kv. Chains run 256 wide
                        # (each in its own rotating psum bank) so the
                        # sigmoid/mult/store pipeline after the very last
                        # matmul handles 4x less data.
                        TR = TW // 2
                        wrt = wfp.tile([P, CT * P], BF16, tag="w1")
                        _dma_chunked(nc, wrt, wr[c], CT * P, 512)
                        yo = None
                        for s in range(CAP // TR):
                            sub = slice(s * TR, (s + 1) * TR)
                            pr = psr.tile([P, TW], F32, tag="psr")
                            for k in range(CT):
                                nc.tensor.matmul(
                                    pr[:, :TR], wrt[:, k * P:(k + 1) * P],
                                    xrt[k][:, sub],
                                    start=(k == 0), stop=(k == CT - 1),
                                )
                            rm = tmpp.tile([P, TR], BF16, tag="rm")
                            nc.scalar.activation(
                                rm[:], pr[:, :TR],
                                mybir.ActivationFunctionType.Sigmoid,
                            )
                            # pair two 256-wide results into one 512-wide
                            # store: full 1KB/partition lines keep the
                            # tail-critical store fast
                            if s % 2 == 0:
                                yo = outp.tile([P, TW], BF16, tag="yo")
                            nc.vector.tensor_tensor(
                                out=yo[:, (s % 2) * TR:(s % 2 + 1) * TR],
                                in0=kvt[c][:, sub], in1=rm[:],
                                op=mybir.AluOpType.mult,
                            )
                            if s % 2 == 1:
                                nc.sync.dma_start(
                                    yout[c, :, (s - 1) * TR:(s + 1) * TR],
                                    yo[:],
                                )

    nc.compile()
    return nc


def _routing(token_ids: np.ndarray):
    """Token -> (per-core global token list [E, CAP], per-core keep mask)."""
    tid = token_ids.reshape(N).astype(np.int64)
    eidx = (tid * HASH_PRIME) % E
    order = np.argsort(eidx, kind="stable")  # FIFO within expert
    counts = np.bincount(eidx, minlength=E)
    starts = np.zeros(E + 1, np.int64)
    np.cumsum(counts, out=starts[1:])

    token_lists = np.empty((E, CAP), np.int64)
    masks = np.zeros((E, CAP), np.float32)
    dropped = []
    fill_needed = []
    for e in range(E):
        grp = order[starts[e]:starts[e + 1]]
        nk = min(len(grp), CAP)
        token_lists[e, :nk] = grp[:nk]
        masks[e, :nk] = 1.0
        dropped.append(grp[CAP:])
        fill_needed.append(CAP - nk)
    dropped = (
        np.concatenate(dropped) if dropped else np.empty(0, np.int64)
    )
    pos = 0
    for e in range(E):
        need = fill_needed[e]
        if need:
            token_lists[e, CAP - need:] = dropped[pos:pos + need]
            pos += need
    assert pos == len(dropped)
    return token_lists, masks


def _tile_first(W, mt):
    """[C, M] -> [mt, P, CT*P] with w[m][p][k*P+q] = W[k*P+p, m*P+q]."""
    ct = W.shape[0] // P
    return np.ascontiguousarray(
        W.reshape(ct, P, mt, P).transpose(2, 1, 0, 3).reshape(mt, P, ct * P)
    )


def _tile_second(W, ct_out):
    """[K, M] -> [ct_out, P, KT*P] with w[m][p][k*P+q] = W[k*P+p, m*P+q]."""
    kt = W.shape[0] // P
    return np.ascontiguousarray(
        W.reshape(kt, P, ct_out, P).transpose(2, 1, 0, 3).reshape(ct_out, P, kt * P)
    )


def kernel(x, shift_state, token_ids, time_maa_k, time_maa_r, Wk, Wv, Wr, Wek, Wev):
    global _COMPILED
    if _COMPILED is None:
        _COMPILED = _build()
    nc = _COMPILED

    x = np.asarray(x, np.float32)
    shift_state = np.asarray(shift_state, np.float32)
    token_lists, masks = _routing(np.asarray(token_ids))

    xf = x.reshape(N, C)
    xprev_f = np.empty_like(xf)
    xprev_f[1:] = xf[:-1]
    xprev_f[np.arange(B) * T] = shift_state

    # token shift on host: xk/xr = x + (xprev - x) * maa
    dx = xprev_f - xf
    maak = np.asarray(time_maa_k, np.float32)
    maar = np.asarray(time_maa_r, np.float32)
    xk_full = xf + dx * maak
    xr_full = xf + dx * maar

    bf = ml_dtypes.bfloat16
    wk_t = _tile_first(np.asarray(Wk, np.float32), MT_D).astype(bf)
    wr_t = _tile_first(np.asarray(Wr, np.float32), CT).astype(bf)
    wv_t = _tile_second(np.asarray(Wv, np.float32), CT).astype(bf)
    Wek = np.asarray(Wek, np.float32)
    Wev = np.asarray(Wev, np.float32)

    def ctmajor(rows):  # [CAP, C] -> [CT, P, CAP] bf16
        return np.ascontiguousarray(rows.T.reshape(CT, P, CAP)).astype(bf)

    in_maps = []
    for e in range(E):
        L = token_lists[e]
        in_maps.append(dict(
            xk=ctmajor(xk_full[L]),
            xr=ctmajor(xr_full[L]),
            maskd=np.ascontiguousarray(
                np.broadcast_to(masks[e], (P, CAP))
            ).astype(bf),
            wk=wk_t,
            wv=wv_t,
            wr=wr_t,
            wek=_tile_first(Wek[e], MT_E).astype(bf),
            wev=_tile_second(Wev[e], CT).astype(bf),
        ))

    res = run_bass_kernel_spmd(
        nc, in_maps, core_ids=list(range(E)),
        trace=bool(os.environ.get("KERNEL_TRACE")),
    )
    global LAST_RESULTS
    LAST_RESULTS = res

    y = np.empty((N, C), np.float32)
    for e in range(E):
        y[token_lists[e]] = (
            res.results[e]["y"].reshape(C, CAP).T.astype(np.float32)
        )
    return y.reshape(B, T, C)
